# revision 22
# baseline (speedup 1.0000x reference)
"""AnchorTarget kernel for 8 TRN2 NeuronCores (Bass/Tile).

Strategy: shard the anchor dimension N=273600 across 8 cores (34200 each,
padded to 268*128=34304); each core's anchors cover a contiguous 19-row
y-band of the feature grid. gt_boxes (100 boxes) are replicated, and each
core COMPACTS on-device the boxes that can overlap its band (box 0 is
always kept first so all-zero-IoU rows resolve to gt[0] like jnp.argmax)
into G2=48 slots via a prefix-sum rank (triangular-ones PE matmul) and a
one-hot selection matrix. The hot loop then computes IoU, first-index
argmax (reverse-iota trick) and the matched-gt gather (PE transpose +
matmul against a per-slot table [gcx, gcy, ln gw, ln gh, cls+1]) on
[128, 48] tiles, with per-op engine assignment balanced across
DVE/ACT/Pool and group-batched reductions.
"""

import sys

for _p in ("/opt/trn_rl_repo",):
    if _p not in sys.path:
        sys.path.insert(0, _p)

import numpy as np

import concourse.bass as bass
import concourse.bacc as bacc
import concourse.mybir as mybir
from concourse import tile, masks
from concourse.bass_utils import run_bass_kernel_spmd

F32 = mybir.dt.float32
ALU = mybir.AluOpType
ACTF = mybir.ActivationFunctionType
AX = mybir.AxisListType

STRIDE = 8
ANCHOR_SIZE = 32
FW, FH = 200, 152
G = 100
G2 = 48                    # compacted gt slots per core
A = 9
N = FW * FH * A            # 273600
NCORES = 8
NA = N // NCORES           # 34200 anchors per core
P = 128
NT = (NA + P - 1) // P     # 268 tiles per core
NAP = NT * P               # 34304 padded per core

RATIOS = np.array([0.5, 1.0, 2.0])
SCALES = np.array([2.0 ** 0, 2.0 ** (1.0 / 3.0), 2.0 ** (2.0 / 3.0)])

KNOBS = {"onehot": 2, "y_dve_n": 3, "wbufs": 8, "sbufs": 6, "skew": 3,
         "sy_eng": "P", "iw_eng": "P", "ih_eng": "P", "inter_eng": "P",
         "iou_eng": "P", "tbmax": 8}


# ---------------------------------------------------------------- host anchors
def _whctrs(a):
    w = a[2] - a[0] + 1.0
    h = a[3] - a[1] + 1.0
    xc = a[0] + 0.5 * (w - 1.0)
    yc = a[1] + 0.5 * (h - 1.0)
    return w, h, xc, yc


def _mkanchors(ws, hs, xc, yc):
    ws = ws[:, None]
    hs = hs[:, None]
    return np.hstack([xc - 0.5 * (ws - 1.0), yc - 0.5 * (hs - 1.0),
                      xc + 0.5 * (ws - 1.0), yc + 0.5 * (hs - 1.0)])


def _generate_anchors(base_size):
    base = np.array([0.0, 0.0, base_size - 1.0, base_size - 1.0])
    w, h, xc, yc = _whctrs(base)
    size_ratios = (w * h) / RATIOS
    ws = np.round(np.sqrt(size_ratios))
    hs = np.round(ws * RATIOS)
    ratio_anchors = _mkanchors(ws, hs, xc, yc)
    out = []
    for i in range(ratio_anchors.shape[0]):
        w, h, xc, yc = _whctrs(ratio_anchors[i])
        out.append(_mkanchors(w * SCALES, h * SCALES, xc, yc))
    return np.vstack(out).astype(np.float32)


def _shift(fw, fh, stride, anchors):
    sx = np.arange(fw) * stride
    sy = np.arange(fh) * stride
    sx, sy = np.meshgrid(sx, sy)
    shifts = np.stack([sx.ravel(), sy.ravel(), sx.ravel(), sy.ravel()], axis=1)
    alla = anchors[None, :, :] + shifts[:, None, :].astype(np.float32)
    return alla.reshape(-1, 4).astype(np.float32)


# ---------------------------------------------------------------- device graph
def build_nc(reps: int = 1):
    kn = KNOBS
    nc = bacc.Bacc("TRN2", target_bir_lowering=False, debug=False,
                   num_devices=NCORES)

    anc_ext = nc.dram_tensor("anc", [P, NT * 4], F32, kind="ExternalInput")
    gt_ext = nc.dram_tensor("gt", [G, 5], F32, kind="ExternalInput")
    band_ext = nc.dram_tensor("band", [G, 2], F32, kind="ExternalInput")
    out_ext = nc.dram_tensor("out", [P, NT * 5], F32, kind="ExternalOutput")

    with tile.TileContext(nc) as tc:
        from contextlib import ExitStack
        ctx = ExitStack()
        with ctx:
            cp = ctx.enter_context(tc.tile_pool(name="const", bufs=1))
            wp = ctx.enter_context(
                tc.tile_pool(name="work", bufs=kn["wbufs"]))
            pp = ctx.enter_context(tc.tile_pool(name="psum", bufs=3,
                                                space="PSUM"))

            # persistent buffers
            anc_s = cp.tile([P, NT * 4], F32)
            outbuf = cp.tile([P, NT * 5], F32)
            gath = cp.tile([P, NT * 5], F32)
            minv_all = cp.tile([P, NT], F32)
            rmax_all = cp.tile([P, NT], F32)
            onebias_all = cp.tile([P, NT], F32)
            gt_s = cp.tile([G, 5], F32)
            band_s = cp.tile([G, 2], F32)
            identity = cp.tile([P, P], F32)
            ltri = cp.tile([G, G], F32)
            ones = cp.tile([1, P], F32)
            slotids = cp.tile([P, G2], F32)
            gtv = cp.tile([G2, 5], F32)
            gt_c = cp.tile([G2, 5], F32)
            rows4 = cp.tile([1, 4 * G2], F32)
            dummyv = cp.tile([1, 4], F32)
            keep = cp.tile([G, 1], F32)
            rank = cp.tile([G, 1], F32)
            rank_m = cp.tile([G, 1], F32)
            notf = cp.tile([1, G2], F32)
            st_sel = cp.tile([G, G2], F32)
            gx1b = cp.tile([P, G2], F32)
            gy1b = cp.tile([P, G2], F32)
            gx2b = cp.tile([P, G2], F32)
            gy2b = cp.tile([P, G2], F32)
            area_gb = cp.tile([P, G2], F32)
            revp_b = cp.tile([P, G2], F32)
            # per-anchor batched quantities
            ew_all = cp.tile([P, NT], F32)
            eh_all = cp.tile([P, NT], F32)
            area_a = cp.tile([P, NT], F32)
            ecx_all = cp.tile([P, NT], F32)
            ecy_all = cp.tile([P, NT], F32)
            iew_all = cp.tile([P, NT], F32)
            ieh_all = cp.tile([P, NT], F32)
            lew_all = cp.tile([P, NT], F32)
            leh_all = cp.tile([P, NT], F32)
            nay1_all = cp.tile([P, NT], F32)

            # input DMAs
            nc.sync.dma_start(gt_s[:], gt_ext.ap())
            nc.sync.dma_start(band_s[:], band_ext.ap())

            # constants: identity, lower-tri ones, ones row, slot iota
            masks.make_identity(nc, identity[:])
            nc.gpsimd.memset(ltri[:], 0.0)
            nc.gpsimd.affine_select(
                out=ltri[:], in_=ltri[:], compare_op=ALU.is_gt, fill=1.0,
                base=0, pattern=[[-1, G]], channel_multiplier=1)
            nc.gpsimd.memset(ones[:], 1.0)
            slot_i = cp.tile([P, G2], mybir.dt.int32)
            nc.gpsimd.iota(slot_i[:], pattern=[[1, G2]], base=0,
                           channel_multiplier=0)
            nc.vector.tensor_copy(slotids[:], slot_i[:])
            revp_i = cp.tile([P, G2], mybir.dt.int32)
            nc.gpsimd.iota(revp_i[:], pattern=[[-1, G2]], base=G2,
                           channel_multiplier=0)
            nc.vector.tensor_copy(revp_b[:], revp_i[:])
            nc.gpsimd.memset(dummyv[:, 0:2], -100000.0)
            nc.gpsimd.memset(dummyv[:, 2:4], -99999.0)

            # ---- gt band compaction ----
            gy1, gy2 = gt_s[:, 1:2], gt_s[:, 3:4]
            blo, bhi = band_s[:, 0:1], band_s[:, 1:2]
            m1 = cp.tile([G, 1], F32)
            m2 = cp.tile([G, 1], F32)
            nc.vector.tensor_tensor(m1[:], gy2, bhi, op=ALU.min)
            nc.vector.tensor_tensor(m2[:], gy1, blo, op=ALU.max)
            k0 = cp.tile([G, 1], F32)
            nc.vector.scalar_tensor_tensor(k0[:], m1[:], 1.0, m2[:],
                                           op0=ALU.add, op1=ALU.subtract)
            nc.vector.tensor_scalar(keep[:], k0[:], 0.0, None, op0=ALU.is_gt)
            nc.gpsimd.memset(keep[0:1, :], 1.0)
            # inclusive prefix sum of keep via lower-triangular ones
            ps_rank = pp.tile([G, 1], F32, tag="setup", bufs=2, name="ps_rank")
            nc.tensor.matmul(ps_rank[:], ltri[:], keep[:], start=True,
                             stop=True)
            nc.scalar.copy(rank[:], ps_rank[:])
            # slot = rank-1 for kept, >=999 for dropped
            a999 = cp.tile([G, 1], F32)
            nc.vector.tensor_scalar_add(a999[:], rank[:], 999.0)
            nc.vector.scalar_tensor_tensor(rank_m[:], keep[:], -1000.0,
                                           a999[:], op0=ALU.mult, op1=ALU.add)
            # selection matrix [g, slot] and unfilled-slot row
            nc.vector.tensor_scalar(st_sel[:], slotids[0:G, :], rank_m[:],
                                    None, op0=ALU.is_equal)
            ones100 = cp.tile([G, 1], F32)
            nc.gpsimd.memset(ones100[:], 1.0)
            ps_cnt = pp.tile([1, 1], F32, tag="setup", bufs=2, name="ps_cnt")
            nc.tensor.matmul(ps_cnt[:], keep[:], ones100[:], start=True,
                             stop=True)
            cnt = cp.tile([1, 1], F32)
            nc.scalar.copy(cnt[:], ps_cnt[:])
            nc.vector.tensor_scalar(notf[:], slotids[0:1, :], cnt[:, 0:1],
                                    None, op0=ALU.is_ge)
            # compacted gt rows [G2, 5] (+dummy box into unfilled slots)
            ps_gtc = pp.tile([G2, 5], F32, tag="setup", bufs=2, name="ps_gtc")
            nc.tensor.matmul(ps_gtc[:], st_sel[:], gt_s[:], start=True,
                             stop=False)
            dummy5 = cp.tile([1, 5], F32)
            nc.vector.tensor_copy(dummy5[:, 0:4], dummyv[:])
            nc.gpsimd.memset(dummy5[:, 4:5], 0.0)
            nc.tensor.matmul(ps_gtc[:], notf[:], dummy5[:], start=False,
                             stop=True)
            nc.scalar.copy(gt_c[:], ps_gtc[:])
            # compacted coord rows [1, 4*G2] then broadcast to [128, G2]
            ps_rows = pp.tile([1, 4 * G2], F32, tag="setup", bufs=2,
                              name="ps_rows")
            for r in range(4):
                nc.tensor.matmul(ps_rows[:, r * G2:(r + 1) * G2],
                                 gt_s[:, r:r + 1], st_sel[:], start=True,
                                 stop=False)
                nc.tensor.matmul(ps_rows[:, r * G2:(r + 1) * G2],
                                 dummyv[:, r:r + 1], notf[:], start=False,
                                 stop=True)
            nc.scalar.copy(rows4[:], ps_rows[:])
            for r, dst in enumerate([gx1b, gy1b, gx2b, gy2b]):
                ps = pp.tile([P, G2], F32, tag="setup", bufs=2, name="psbc")
                nc.tensor.matmul(ps[:], ones[:],
                                 rows4[:, r * G2:(r + 1) * G2],
                                 start=True, stop=True)
                nc.scalar.copy(dst[:], ps[:])

            # area_g broadcast: (gx2-gx1+1)*(gy2-gy1+1)
            wg = cp.tile([P, G2], F32)
            hg = cp.tile([P, G2], F32)
            nc.vector.scalar_tensor_tensor(wg[:], gx1b[:], -1.0, gx2b[:],
                                           op0=ALU.mult, op1=ALU.add)
            nc.vector.tensor_scalar_add(wg[:], wg[:], 1.0)
            nc.vector.scalar_tensor_tensor(hg[:], gy1b[:], -1.0, gy2b[:],
                                           op0=ALU.mult, op1=ALU.add)
            nc.vector.tensor_scalar_add(hg[:], hg[:], 1.0)
            nc.vector.tensor_tensor(area_gb[:], wg[:], hg[:], op=ALU.mult)

            # per-slot gather table: [gcx, gcy, ln gw, ln gh, cls+1]
            x1, y1 = gt_c[:, 0:1], gt_c[:, 1:2]
            x2, y2 = gt_c[:, 2:3], gt_c[:, 3:4]
            cls = gt_c[:, 4:5]
            gw = cp.tile([G2, 1], F32)
            gh = cp.tile([G2, 1], F32)
            nc.vector.scalar_tensor_tensor(gw[:], x1, -1.0, x2,
                                           op0=ALU.mult, op1=ALU.add)
            nc.vector.tensor_scalar_add(gw[:], gw[:], 1.0)
            nc.vector.scalar_tensor_tensor(gh[:], y1, -1.0, y2,
                                           op0=ALU.mult, op1=ALU.add)
            nc.vector.tensor_scalar_add(gh[:], gh[:], 1.0)
            nc.vector.scalar_tensor_tensor(gtv[:, 0:1], gw[:], 0.5, x1,
                                           op0=ALU.mult, op1=ALU.add)
            nc.vector.scalar_tensor_tensor(gtv[:, 1:2], gh[:], 0.5, y1,
                                           op0=ALU.mult, op1=ALU.add)
            nc.scalar.activation(gtv[:, 2:3], gw[:], ACTF.Ln)
            nc.scalar.activation(gtv[:, 3:4], gh[:], ACTF.Ln)
            nc.vector.tensor_scalar_add(gtv[:, 4:5], cls, 1.0)

            # ---- repeated body (reps>1 used only for slope timing) ----
            for _rep in range(reps):
              nc.sync.dma_start(anc_s[:], anc_ext.ap())
              av = anc_s[:].rearrange("p (t c) -> p t c", c=4)
              NCH = 4
              CH = NT // NCH
              for c_ in range(NCH):
                  cs = slice(c_ * CH, NT if c_ == NCH - 1 else (c_ + 1) * CH)
                  ax1v, ay1v = av[:, cs, 0], av[:, cs, 1]
                  ax2v, ay2v = av[:, cs, 2], av[:, cs, 3]
                  nc.vector.scalar_tensor_tensor(ew_all[:, cs], ax1v, -1.0,
                                                 ax2v, op0=ALU.mult,
                                                 op1=ALU.add)
                  nc.vector.tensor_scalar_add(ew_all[:, cs], ew_all[:, cs],
                                              1.0)
                  nc.vector.scalar_tensor_tensor(eh_all[:, cs], ay1v, -1.0,
                                                 ay2v, op0=ALU.mult,
                                                 op1=ALU.add)
                  nc.vector.tensor_scalar_add(eh_all[:, cs], eh_all[:, cs],
                                              1.0)
                  nc.gpsimd.tensor_tensor(area_a[:, cs], ew_all[:, cs],
                                          eh_all[:, cs], op=ALU.mult)
                  nc.vector.scalar_tensor_tensor(ecx_all[:, cs],
                                                 ew_all[:, cs], 0.5, ax1v,
                                                 op0=ALU.mult, op1=ALU.add)
                  nc.vector.scalar_tensor_tensor(ecy_all[:, cs],
                                                 eh_all[:, cs], 0.5, ay1v,
                                                 op0=ALU.mult, op1=ALU.add)
                  nc.vector.tensor_scalar_mul(nay1_all[:, cs], ay1v, -1.0)
                  nc.vector.reciprocal(iew_all[:, cs], ew_all[:, cs])
                  nc.vector.reciprocal(ieh_all[:, cs], eh_all[:, cs])
                  nc.scalar.activation(lew_all[:, cs], ew_all[:, cs], ACTF.Ln)
                  nc.scalar.activation(leh_all[:, cs], eh_all[:, cs], ACTF.Ln)

              # ------------------------------------------------ main loop
              TBMAX = kn["tbmax"]
              groups = []
              t0 = 0
              while t0 < NT:
                  tb = min(TBMAX, NT - t0)
                  groups.append((t0, tb))
                  t0 += tb
              def phase1(gt0, TB):
                  st = {}
                  st["ry1_st"] = wp.tile([P, TBMAX, G2], F32, tag="ry1_st",
                                         name="ry1_st", bufs=kn["sbufs"])
                  st["ry2_st"] = wp.tile([P, TBMAX, G2], F32, tag="ry2_st",
                                         name="ry2_st", bufs=kn["sbufs"])
                  st["iw0_st"] = wp.tile([P, TBMAX, G2], F32, tag="iw0_st",
                                         name="iw0_st", bufs=kn["sbufs"])
                  st["sy_st"] = wp.tile([P, TBMAX, G2], F32, tag="sy_st",
                                        name="sy_st", bufs=kn["sbufs"])
                  st["ih_st"] = wp.tile([P, TBMAX, G2], F32, tag="ih_st",
                                        name="ih_st", bufs=kn["sbufs"])
                  st["inter_st"] = wp.tile([P, TBMAX, G2], F32,
                                           tag="inter_st", name="inter_st",
                                           bufs=kn["sbufs"])
                  st["union_st"] = wp.tile([P, TBMAX, G2], F32,
                                           tag="union_st", name="union_st",
                                           bufs=kn["sbufs"])
                  st["iou_st"] = wp.tile([P, TBMAX, G2], F32, tag="iou_st",
                                         name="iou_st", bufs=kn["sbufs"])
                  st["mrev_st"] = wp.tile([P, TBMAX, G2], F32, tag="mrev_st",
                                          name="mrev_st", bufs=kn["sbufs"])
                  ry1_st, ry2_st = st["ry1_st"], st["ry2_st"]
                  iw0_st, sy_st, ih_st = st["iw0_st"], st["sy_st"], st["ih_st"]
                  inter_st, union_st = st["inter_st"], st["union_st"]
                  iou_st = st["iou_st"]
                  ydn = min(kn["y_dve_n"], TB)
                  st["ydn"] = ydn
                  for j in range(TB):
                      t = gt0 + j
                      ax1 = anc_s[:, 4 * t + 0:4 * t + 1]
                      ay1 = anc_s[:, 4 * t + 1:4 * t + 2]
                      ax2 = anc_s[:, 4 * t + 2:4 * t + 3]
                      ay2 = anc_s[:, 4 * t + 3:4 * t + 4]
                      if j < ydn:
                          t2y = wp.tile([P, G2], F32, tag="t2y", name="t2y")
                          nc.vector.tensor_scalar(t2y[:], gy1b[:], ay1, 1.0,
                                                  op0=ALU.max,
                                                  op1=ALU.subtract)
                          nc.vector.scalar_tensor_tensor(
                              ih_st[:, j, :], gy2b[:], ay2, t2y[:],
                              op0=ALU.min, op1=ALU.subtract)
                      else:
                          # y axis on ACT: two hinge terms
                          nc.scalar.activation(ry1_st[:, j, :], gy1b[:],
                                               ACTF.Relu,
                                               bias=nay1_all[:, t:t + 1])
                          nc.scalar.activation(ry2_st[:, j, :], gy2b[:],
                                               ACTF.Relu, bias=ay2,
                                               scale=-1.0)
                      # x axis on DVE
                      t2x = wp.tile([P, G2], F32, tag="t2x", name="t2x")
                      nc.vector.tensor_scalar(t2x[:], gx1b[:], ax1, 1.0,
                                              op0=ALU.max, op1=ALU.subtract)
                      nc.vector.scalar_tensor_tensor(iw0_st[:, j, :], gx2b[:],
                                                     ax2, t2x[:], op0=ALU.min,
                                                     op1=ALU.subtract)
                  bsl = (slice(None), slice(0, TB), slice(None))
                  asl = (slice(None), slice(ydn, TB), slice(None))
                  csl = (slice(None), slice(0, ydn), slice(None))

                  def _eng(key):
                      return nc.gpsimd if kn[key] == "P" else nc.vector
                  # batched: sy = ry1 + ry2 (ACT-y slices), iw clamp in place
                  if ydn < TB:
                      _eng("sy_eng").tensor_tensor(sy_st[asl], ry1_st[asl],
                                                   ry2_st[asl], op=ALU.add)
                  _eng("iw_eng").tensor_scalar(iw0_st[bsl], iw0_st[bsl], 0.0,
                                               None, op0=ALU.max)
                  for j in range(ydn, TB):
                      t = gt0 + j
                      # ih = relu(eh - sy) per tile on ACT
                      nc.scalar.activation(ih_st[:, j, :], sy_st[:, j, :],
                                           ACTF.Relu,
                                           bias=eh_all[:, t:t + 1],
                                           scale=-1.0)
                  if ydn:
                      # clamp ih for the DVE-y slices
                      _eng("ih_eng").tensor_scalar(ih_st[csl], ih_st[csl],
                                                   0.0, None, op0=ALU.max)
                  # batched: inter = iw * ih
                  _eng("inter_eng").tensor_tensor(inter_st[bsl], iw0_st[bsl],
                                                  ih_st[bsl], op=ALU.mult)
                  for j in range(TB):
                      t = gt0 + j
                      aa = area_a[:, t:t + 1]
                      nc.vector.scalar_tensor_tensor(union_st[:, j, :],
                                                     area_gb[:], aa,
                                                     inter_st[:, j, :],
                                                     op0=ALU.add,
                                                     op1=ALU.subtract)
                  # batched: urec = 1/union (DVE, in place), iou
                  nc.vector.reciprocal(union_st[bsl], union_st[bsl])
                  _eng("iou_eng").tensor_tensor(iou_st[bsl], inter_st[bsl],
                                                union_st[bsl], op=ALU.mult)
                  return st

              def phase2(gt0, TB, st):
                  bsl = (slice(None), slice(0, TB), slice(None))
                  iou_st, mrev_st = st["iou_st"], st["mrev_st"]
                  # batched max-iou over the group
                  nc.vector.tensor_reduce(minv_all[:, gt0:gt0 + TB],
                                          iou_st[bsl], axis=AX.X, op=ALU.max)
                  for j in range(TB):
                      t = gt0 + j
                      nc.vector.scalar_tensor_tensor(
                          mrev_st[:, j, :], iou_st[:, j, :],
                          minv_all[:, t:t + 1], revp_b[:],
                          op0=ALU.is_equal, op1=ALU.mult)
                  nc.vector.tensor_reduce(rmax_all[:, gt0:gt0 + TB],
                                          mrev_st[bsl], axis=AX.X, op=ALU.max)
                  # onebias = 1 - rmax (tiny, Pool)
                  nc.gpsimd.tensor_scalar(onebias_all[:, gt0:gt0 + TB],
                                          rmax_all[:, gt0:gt0 + TB],
                                          -1.0, 1.0, op0=ALU.mult,
                                          op1=ALU.add)
                  ohT4 = wp.tile([G2, TBMAX * P], F32, tag="ohT4",
                                 name="ohT4", bufs=2)
                  gps4 = pp.tile([P, TBMAX * 5], F32, tag="gps4", name="gps4")
                  for h0 in range(0, TB, 4):
                      hn = min(4, TB - h0)
                      psT4 = pp.tile([G2, 4 * P], F32, tag="psT4",
                                     name="psT4")
                      for j in range(h0, h0 + hn):
                          t = gt0 + j
                          onehot = wp.tile([P, G2], F32, tag="onehot",
                                           name="onehot")
                          if kn["onehot"] == 0 or (kn["onehot"] == 2
                                                   and t % 2 == 0):
                              nc.vector.tensor_scalar(onehot[:],
                                                      mrev_st[:, j, :],
                                                      rmax_all[:, t:t + 1],
                                                      None, op0=ALU.is_equal)
                          else:
                              # exact: mrev integer-valued, rmax its max
                              nc.scalar.activation(
                                  onehot[:], mrev_st[:, j, :], ACTF.Relu,
                                  bias=onebias_all[:, t:t + 1])
                          nc.tensor.transpose(psT4[:, (j - h0) * P:
                                                   (j - h0 + 1) * P],
                                              onehot[:], identity[:])
                      nc.scalar.copy(ohT4[:, h0 * P:(h0 + hn) * P],
                                     psT4[:, 0:hn * P])
                  for j in range(TB):
                      nc.tensor.matmul(gps4[:, j * 5:(j + 1) * 5],
                                       ohT4[:, j * P:(j + 1) * P],
                                       gtv[:, 0:5], start=True, stop=True)
                  nc.scalar.copy(gath[:, 5 * gt0:5 * (gt0 + TB)],
                                 gps4[:, 0:TB * 5])

              # software-pipelined emission: phase1(g+1) before phase2(g)
              pend = []
              for (gt0, TB) in groups:
                  pend.append((gt0, TB, phase1(gt0, TB)))
                  if len(pend) > kn.get("skew", 1):
                      g0, tb0, st0 = pend.pop(0)
                      phase2(g0, tb0, st0)
              for (g0, tb0, st0) in pend:
                  phase2(g0, tb0, st0)
              # ------------------------------------------------ epilogue
              gv = gath[:].rearrange("p (t c) -> p t c", c=5)
              ob = outbuf[:].rearrange("p (t c) -> p t c", c=5)
              tmp1 = cp.tile([P, NT], F32)
              tmp2 = cp.tile([P, NT], F32)
              lm1 = cp.tile([P, NT], F32)
              lm2 = cp.tile([P, NT], F32)
              for c_ in range(NCH):
                  cs = slice(c_ * CH, NT if c_ == NCH - 1 else (c_ + 1) * CH)
                  # dx, dy
                  nc.vector.tensor_sub(tmp1[:, cs], gv[:, cs, 0],
                                       ecx_all[:, cs])
                  nc.vector.tensor_tensor(ob[:, cs, 1], tmp1[:, cs],
                                          iew_all[:, cs], op=ALU.mult)
                  nc.vector.tensor_sub(tmp2[:, cs], gv[:, cs, 1],
                                       ecy_all[:, cs])
                  nc.vector.tensor_tensor(ob[:, cs, 2], tmp2[:, cs],
                                          ieh_all[:, cs], op=ALU.mult)
                  # dw, dh
                  nc.vector.tensor_sub(ob[:, cs, 3], gv[:, cs, 2],
                                       lew_all[:, cs])
                  nc.vector.tensor_sub(ob[:, cs, 4], gv[:, cs, 3],
                                       leh_all[:, cs])
                  # labels: pos*(cls+1) + neg - 1
                  nc.vector.scalar_tensor_tensor(lm1[:, cs], minv_all[:, cs],
                                                 0.5, gv[:, cs, 4],
                                                 op0=ALU.is_ge, op1=ALU.mult)
                  nc.vector.scalar_tensor_tensor(lm2[:, cs], minv_all[:, cs],
                                                 0.4, lm1[:, cs],
                                                 op0=ALU.is_lt, op1=ALU.add)
                  nc.vector.tensor_scalar_add(ob[:, cs, 0], lm2[:, cs], -1.0)
              nc.sync.dma_start(out_ext.ap(), outbuf[:])

    nc.compile()
    return nc


_NC = {}


def _get_nc(reps: int = 1):
    if reps not in _NC:
        _NC[reps] = build_nc(reps)
    return _NC[reps]


def make_in_maps(gt_boxes):
    anchors = _shift(FW, FH, STRIDE, _generate_anchors(ANCHOR_SIZE))
    gt = np.asarray(gt_boxes, dtype=np.float32)[0]          # [G, 5]
    in_maps = []
    for c in range(NCORES):
        shard = anchors[c * NA:(c + 1) * NA]
        pad = np.zeros((NAP - NA, 4), dtype=np.float32)
        sh = np.concatenate([shard, pad], axis=0)           # [NAP, 4]
        anc = np.ascontiguousarray(
            sh.reshape(NT, P, 4).transpose(1, 0, 2).reshape(P, NT * 4))
        lo = np.float32(shard[:, 1].min() - 1.0)
        hi = np.float32(shard[:, 3].max() + 1.0)
        band = np.tile(np.array([[lo, hi]], dtype=np.float32), (G, 1))
        in_maps.append({"anc": anc, "gt": gt, "band": band})
    return anchors, in_maps


def kernel(gt_boxes, fw, fh):
    assert int(fw) == FW and int(fh) == FH
    anchors, in_maps = make_in_maps(gt_boxes)
    nc = _get_nc()
    res = run_bass_kernel_spmd(nc, in_maps, core_ids=list(range(NCORES)))
    parts = []
    for c in range(NCORES):
        o = res.results[c]["out"]                           # [P, NT*5]
        o = o.reshape(P, NT, 5).transpose(1, 0, 2).reshape(NAP, 5)[:NA]
        parts.append(o)
    full = np.concatenate(parts, axis=0)                    # [N, 5]
    labels = np.ascontiguousarray(full[:, 0])[None]
    targets = np.ascontiguousarray(full[:, 1:5])[None]
    return labels, targets, anchors[None]


if __name__ == "__main__":
    gt = np.random.rand(1, G, 5).astype(np.float32)
    out = kernel(gt_boxes=gt, fw=FW, fh=FH)
    print([o.shape for o in out])


# revision 23
# speedup vs baseline: 1.0756x; 1.0756x over previous
"""AnchorTarget kernel for 8 TRN2 NeuronCores (Bass/Tile).

Strategy: shard the anchor dimension N=273600 across 8 cores (34200 each,
padded to 268*128=34304); each core's anchors cover a contiguous 19-row
y-band of the feature grid. gt_boxes (100 boxes) are replicated, and each
core COMPACTS on-device the boxes that can overlap its band (box 0 is
always kept first so all-zero-IoU rows resolve to gt[0] like jnp.argmax)
into G2=48 slots via a prefix-sum rank (triangular-ones PE matmul) and a
one-hot selection matrix. The hot loop then computes IoU, first-index
argmax (reverse-iota trick) and the matched-gt gather (PE transpose +
matmul against a per-slot table [gcx, gcy, ln gw, ln gh, cls+1]) on
[128, 48] tiles, with per-op engine assignment balanced across
DVE/ACT/Pool and group-batched reductions.
"""

import sys

for _p in ("/opt/trn_rl_repo",):
    if _p not in sys.path:
        sys.path.insert(0, _p)

import numpy as np

import concourse.bass as bass
import concourse.bacc as bacc
import concourse.mybir as mybir
from concourse import tile, masks
from concourse.bass_utils import run_bass_kernel_spmd

F32 = mybir.dt.float32
ALU = mybir.AluOpType
ACTF = mybir.ActivationFunctionType
AX = mybir.AxisListType

STRIDE = 8
ANCHOR_SIZE = 32
FW, FH = 200, 152
G = 100
G2 = 40                    # compacted gt slots per core
A = 9
N = FW * FH * A            # 273600
NCORES = 8
NA = N // NCORES           # 34200 anchors per core
P = 128
NT = (NA + P - 1) // P     # 268 tiles per core
NAP = NT * P               # 34304 padded per core

RATIOS = np.array([0.5, 1.0, 2.0])
SCALES = np.array([2.0 ** 0, 2.0 ** (1.0 / 3.0), 2.0 ** (2.0 / 3.0)])

KNOBS = {"onehot": 2, "y_dve_n": 3, "wbufs": 8, "sbufs": 6, "skew": 3,
         "sy_eng": "P", "iw_eng": "P", "ih_eng": "P", "inter_eng": "P",
         "iou_eng": "P", "tbmax": 8, "ohTcopy": "A"}


# ---------------------------------------------------------------- host anchors
def _whctrs(a):
    w = a[2] - a[0] + 1.0
    h = a[3] - a[1] + 1.0
    xc = a[0] + 0.5 * (w - 1.0)
    yc = a[1] + 0.5 * (h - 1.0)
    return w, h, xc, yc


def _mkanchors(ws, hs, xc, yc):
    ws = ws[:, None]
    hs = hs[:, None]
    return np.hstack([xc - 0.5 * (ws - 1.0), yc - 0.5 * (hs - 1.0),
                      xc + 0.5 * (ws - 1.0), yc + 0.5 * (hs - 1.0)])


def _generate_anchors(base_size):
    base = np.array([0.0, 0.0, base_size - 1.0, base_size - 1.0])
    w, h, xc, yc = _whctrs(base)
    size_ratios = (w * h) / RATIOS
    ws = np.round(np.sqrt(size_ratios))
    hs = np.round(ws * RATIOS)
    ratio_anchors = _mkanchors(ws, hs, xc, yc)
    out = []
    for i in range(ratio_anchors.shape[0]):
        w, h, xc, yc = _whctrs(ratio_anchors[i])
        out.append(_mkanchors(w * SCALES, h * SCALES, xc, yc))
    return np.vstack(out).astype(np.float32)


def _shift(fw, fh, stride, anchors):
    sx = np.arange(fw) * stride
    sy = np.arange(fh) * stride
    sx, sy = np.meshgrid(sx, sy)
    shifts = np.stack([sx.ravel(), sy.ravel(), sx.ravel(), sy.ravel()], axis=1)
    alla = anchors[None, :, :] + shifts[:, None, :].astype(np.float32)
    return alla.reshape(-1, 4).astype(np.float32)


# ---------------------------------------------------------------- device graph
def build_nc(reps: int = 1):
    kn = KNOBS
    nc = bacc.Bacc("TRN2", target_bir_lowering=False, debug=False,
                   num_devices=NCORES)

    anc_ext = nc.dram_tensor("anc", [P, NT * 4], F32, kind="ExternalInput")
    gt_ext = nc.dram_tensor("gt", [G, 5], F32, kind="ExternalInput")
    band_ext = nc.dram_tensor("band", [G, 2], F32, kind="ExternalInput")
    out_ext = nc.dram_tensor("out", [P, NT * 5], F32, kind="ExternalOutput")

    with tile.TileContext(nc) as tc:
        from contextlib import ExitStack
        ctx = ExitStack()
        with ctx:
            cp = ctx.enter_context(tc.tile_pool(name="const", bufs=1))
            wp = ctx.enter_context(
                tc.tile_pool(name="work", bufs=kn["wbufs"]))
            pp = ctx.enter_context(tc.tile_pool(name="psum", bufs=3,
                                                space="PSUM"))

            # persistent buffers
            anc_s = cp.tile([P, NT * 4], F32)
            outbuf = cp.tile([P, NT * 5], F32)
            gath = cp.tile([P, NT * 5], F32)
            minv_all = cp.tile([P, NT], F32)
            rmax_all = cp.tile([P, NT], F32)
            onebias_all = cp.tile([P, NT], F32)
            gt_s = cp.tile([G, 5], F32)
            band_s = cp.tile([G, 2], F32)
            identity = cp.tile([P, P], F32)
            ltri = cp.tile([G, G], F32)
            ones = cp.tile([1, P], F32)
            slotids = cp.tile([P, G2], F32)
            gtv = cp.tile([G2, 5], F32)
            gt_c = cp.tile([G2, 5], F32)
            rows4 = cp.tile([1, 4 * G2], F32)
            dummyv = cp.tile([1, 4], F32)
            keep = cp.tile([G, 1], F32)
            rank = cp.tile([G, 1], F32)
            rank_m = cp.tile([G, 1], F32)
            notf = cp.tile([1, G2], F32)
            st_sel = cp.tile([G, G2], F32)
            gx1b = cp.tile([P, G2], F32)
            gy1b = cp.tile([P, G2], F32)
            gx2b = cp.tile([P, G2], F32)
            gy2b = cp.tile([P, G2], F32)
            area_gb = cp.tile([P, G2], F32)
            revp_b = cp.tile([P, G2], F32)
            # per-anchor batched quantities
            ew_all = cp.tile([P, NT], F32)
            eh_all = cp.tile([P, NT], F32)
            area_a = cp.tile([P, NT], F32)
            ecx_all = cp.tile([P, NT], F32)
            ecy_all = cp.tile([P, NT], F32)
            iew_all = cp.tile([P, NT], F32)
            ieh_all = cp.tile([P, NT], F32)
            lew_all = cp.tile([P, NT], F32)
            leh_all = cp.tile([P, NT], F32)
            nay1_all = cp.tile([P, NT], F32)

            # input DMAs
            nc.sync.dma_start(gt_s[:], gt_ext.ap())
            nc.sync.dma_start(band_s[:], band_ext.ap())

            # constants: identity, lower-tri ones, ones row, slot iota
            masks.make_identity(nc, identity[:])
            nc.gpsimd.memset(ltri[:], 0.0)
            nc.gpsimd.affine_select(
                out=ltri[:], in_=ltri[:], compare_op=ALU.is_gt, fill=1.0,
                base=0, pattern=[[-1, G]], channel_multiplier=1)
            nc.gpsimd.memset(ones[:], 1.0)
            slot_i = cp.tile([P, G2], mybir.dt.int32)
            nc.gpsimd.iota(slot_i[:], pattern=[[1, G2]], base=0,
                           channel_multiplier=0)
            nc.vector.tensor_copy(slotids[:], slot_i[:])
            revp_i = cp.tile([P, G2], mybir.dt.int32)
            nc.gpsimd.iota(revp_i[:], pattern=[[-1, G2]], base=G2,
                           channel_multiplier=0)
            nc.vector.tensor_copy(revp_b[:], revp_i[:])
            nc.gpsimd.memset(dummyv[:, 0:2], -100000.0)
            nc.gpsimd.memset(dummyv[:, 2:4], -99999.0)

            # ---- gt band compaction ----
            gy1, gy2 = gt_s[:, 1:2], gt_s[:, 3:4]
            blo, bhi = band_s[:, 0:1], band_s[:, 1:2]
            m1 = cp.tile([G, 1], F32)
            m2 = cp.tile([G, 1], F32)
            nc.vector.tensor_tensor(m1[:], gy2, bhi, op=ALU.min)
            nc.vector.tensor_tensor(m2[:], gy1, blo, op=ALU.max)
            k0 = cp.tile([G, 1], F32)
            nc.vector.scalar_tensor_tensor(k0[:], m1[:], 1.0, m2[:],
                                           op0=ALU.add, op1=ALU.subtract)
            nc.vector.tensor_scalar(keep[:], k0[:], 0.0, None, op0=ALU.is_gt)
            nc.gpsimd.memset(keep[0:1, :], 1.0)
            # inclusive prefix sum of keep via lower-triangular ones
            ps_rank = pp.tile([G, 1], F32, tag="setup", bufs=2, name="ps_rank")
            nc.tensor.matmul(ps_rank[:], ltri[:], keep[:], start=True,
                             stop=True)
            nc.scalar.copy(rank[:], ps_rank[:])
            # slot = rank-1 for kept, >=999 for dropped
            a999 = cp.tile([G, 1], F32)
            nc.vector.tensor_scalar_add(a999[:], rank[:], 999.0)
            nc.vector.scalar_tensor_tensor(rank_m[:], keep[:], -1000.0,
                                           a999[:], op0=ALU.mult, op1=ALU.add)
            # selection matrix [g, slot] and unfilled-slot row
            nc.vector.tensor_scalar(st_sel[:], slotids[0:G, :], rank_m[:],
                                    None, op0=ALU.is_equal)
            ones100 = cp.tile([G, 1], F32)
            nc.gpsimd.memset(ones100[:], 1.0)
            ps_cnt = pp.tile([1, 1], F32, tag="setup", bufs=2, name="ps_cnt")
            nc.tensor.matmul(ps_cnt[:], keep[:], ones100[:], start=True,
                             stop=True)
            cnt = cp.tile([1, 1], F32)
            nc.scalar.copy(cnt[:], ps_cnt[:])
            nc.vector.tensor_scalar(notf[:], slotids[0:1, :], cnt[:, 0:1],
                                    None, op0=ALU.is_ge)
            # compacted gt rows [G2, 5] (+dummy box into unfilled slots)
            ps_gtc = pp.tile([G2, 5], F32, tag="setup", bufs=2, name="ps_gtc")
            nc.tensor.matmul(ps_gtc[:], st_sel[:], gt_s[:], start=True,
                             stop=False)
            dummy5 = cp.tile([1, 5], F32)
            nc.vector.tensor_copy(dummy5[:, 0:4], dummyv[:])
            nc.gpsimd.memset(dummy5[:, 4:5], 0.0)
            nc.tensor.matmul(ps_gtc[:], notf[:], dummy5[:], start=False,
                             stop=True)
            nc.scalar.copy(gt_c[:], ps_gtc[:])
            # compacted coord rows [1, 4*G2] then broadcast to [128, G2]
            ps_rows = pp.tile([1, 4 * G2], F32, tag="setup", bufs=2,
                              name="ps_rows")
            for r in range(4):
                nc.tensor.matmul(ps_rows[:, r * G2:(r + 1) * G2],
                                 gt_s[:, r:r + 1], st_sel[:], start=True,
                                 stop=False)
                nc.tensor.matmul(ps_rows[:, r * G2:(r + 1) * G2],
                                 dummyv[:, r:r + 1], notf[:], start=False,
                                 stop=True)
            nc.scalar.copy(rows4[:], ps_rows[:])
            for r, dst in enumerate([gx1b, gy1b, gx2b, gy2b]):
                ps = pp.tile([P, G2], F32, tag="setup", bufs=2, name="psbc")
                nc.tensor.matmul(ps[:], ones[:],
                                 rows4[:, r * G2:(r + 1) * G2],
                                 start=True, stop=True)
                nc.scalar.copy(dst[:], ps[:])

            # area_g broadcast: (gx2-gx1+1)*(gy2-gy1+1)
            wg = cp.tile([P, G2], F32)
            hg = cp.tile([P, G2], F32)
            nc.vector.scalar_tensor_tensor(wg[:], gx1b[:], -1.0, gx2b[:],
                                           op0=ALU.mult, op1=ALU.add)
            nc.vector.tensor_scalar_add(wg[:], wg[:], 1.0)
            nc.vector.scalar_tensor_tensor(hg[:], gy1b[:], -1.0, gy2b[:],
                                           op0=ALU.mult, op1=ALU.add)
            nc.vector.tensor_scalar_add(hg[:], hg[:], 1.0)
            nc.vector.tensor_tensor(area_gb[:], wg[:], hg[:], op=ALU.mult)

            # per-slot gather table: [gcx, gcy, ln gw, ln gh, cls+1]
            x1, y1 = gt_c[:, 0:1], gt_c[:, 1:2]
            x2, y2 = gt_c[:, 2:3], gt_c[:, 3:4]
            cls = gt_c[:, 4:5]
            gw = cp.tile([G2, 1], F32)
            gh = cp.tile([G2, 1], F32)
            nc.vector.scalar_tensor_tensor(gw[:], x1, -1.0, x2,
                                           op0=ALU.mult, op1=ALU.add)
            nc.vector.tensor_scalar_add(gw[:], gw[:], 1.0)
            nc.vector.scalar_tensor_tensor(gh[:], y1, -1.0, y2,
                                           op0=ALU.mult, op1=ALU.add)
            nc.vector.tensor_scalar_add(gh[:], gh[:], 1.0)
            nc.vector.scalar_tensor_tensor(gtv[:, 0:1], gw[:], 0.5, x1,
                                           op0=ALU.mult, op1=ALU.add)
            nc.vector.scalar_tensor_tensor(gtv[:, 1:2], gh[:], 0.5, y1,
                                           op0=ALU.mult, op1=ALU.add)
            nc.scalar.activation(gtv[:, 2:3], gw[:], ACTF.Ln)
            nc.scalar.activation(gtv[:, 3:4], gh[:], ACTF.Ln)
            nc.vector.tensor_scalar_add(gtv[:, 4:5], cls, 1.0)

            # ---- repeated body (reps>1 used only for slope timing) ----
            for _rep in range(reps):
              nc.sync.dma_start(anc_s[:], anc_ext.ap())
              av = anc_s[:].rearrange("p (t c) -> p t c", c=4)
              NCH = 4
              CH = NT // NCH
              for c_ in range(NCH):
                  cs = slice(c_ * CH, NT if c_ == NCH - 1 else (c_ + 1) * CH)
                  ax1v, ay1v = av[:, cs, 0], av[:, cs, 1]
                  ax2v, ay2v = av[:, cs, 2], av[:, cs, 3]
                  nc.vector.scalar_tensor_tensor(ew_all[:, cs], ax1v, -1.0,
                                                 ax2v, op0=ALU.mult,
                                                 op1=ALU.add)
                  nc.vector.tensor_scalar_add(ew_all[:, cs], ew_all[:, cs],
                                              1.0)
                  nc.vector.scalar_tensor_tensor(eh_all[:, cs], ay1v, -1.0,
                                                 ay2v, op0=ALU.mult,
                                                 op1=ALU.add)
                  nc.vector.tensor_scalar_add(eh_all[:, cs], eh_all[:, cs],
                                              1.0)
                  nc.gpsimd.tensor_tensor(area_a[:, cs], ew_all[:, cs],
                                          eh_all[:, cs], op=ALU.mult)
                  nc.vector.scalar_tensor_tensor(ecx_all[:, cs],
                                                 ew_all[:, cs], 0.5, ax1v,
                                                 op0=ALU.mult, op1=ALU.add)
                  nc.vector.scalar_tensor_tensor(ecy_all[:, cs],
                                                 eh_all[:, cs], 0.5, ay1v,
                                                 op0=ALU.mult, op1=ALU.add)
                  nc.vector.tensor_scalar_mul(nay1_all[:, cs], ay1v, -1.0)
                  nc.vector.reciprocal(iew_all[:, cs], ew_all[:, cs])
                  nc.vector.reciprocal(ieh_all[:, cs], eh_all[:, cs])
                  nc.scalar.activation(lew_all[:, cs], ew_all[:, cs], ACTF.Ln)
                  nc.scalar.activation(leh_all[:, cs], eh_all[:, cs], ACTF.Ln)

              # ------------------------------------------------ main loop
              TBMAX = kn["tbmax"]
              groups = []
              t0 = 0
              while t0 < NT:
                  tb = min(TBMAX, NT - t0)
                  groups.append((t0, tb))
                  t0 += tb
              def phase1(gt0, TB):
                  st = {}
                  st["ry1_st"] = wp.tile([P, TBMAX, G2], F32, tag="ry1_st",
                                         name="ry1_st", bufs=kn["sbufs"])
                  st["ry2_st"] = wp.tile([P, TBMAX, G2], F32, tag="ry2_st",
                                         name="ry2_st", bufs=kn["sbufs"])
                  st["iw0_st"] = wp.tile([P, TBMAX, G2], F32, tag="iw0_st",
                                         name="iw0_st", bufs=kn["sbufs"])
                  st["sy_st"] = wp.tile([P, TBMAX, G2], F32, tag="sy_st",
                                        name="sy_st", bufs=kn["sbufs"])
                  st["ih_st"] = wp.tile([P, TBMAX, G2], F32, tag="ih_st",
                                        name="ih_st", bufs=kn["sbufs"])
                  st["inter_st"] = wp.tile([P, TBMAX, G2], F32,
                                           tag="inter_st", name="inter_st",
                                           bufs=kn["sbufs"])
                  st["union_st"] = wp.tile([P, TBMAX, G2], F32,
                                           tag="union_st", name="union_st",
                                           bufs=kn["sbufs"])
                  st["iou_st"] = wp.tile([P, TBMAX, G2], F32, tag="iou_st",
                                         name="iou_st", bufs=kn["sbufs"])
                  st["mrev_st"] = wp.tile([P, TBMAX, G2], F32, tag="mrev_st",
                                          name="mrev_st", bufs=kn["sbufs"])
                  ry1_st, ry2_st = st["ry1_st"], st["ry2_st"]
                  iw0_st, sy_st, ih_st = st["iw0_st"], st["sy_st"], st["ih_st"]
                  inter_st, union_st = st["inter_st"], st["union_st"]
                  iou_st = st["iou_st"]
                  ydn = min(kn["y_dve_n"], TB)
                  st["ydn"] = ydn
                  for j in range(TB):
                      t = gt0 + j
                      ax1 = anc_s[:, 4 * t + 0:4 * t + 1]
                      ay1 = anc_s[:, 4 * t + 1:4 * t + 2]
                      ax2 = anc_s[:, 4 * t + 2:4 * t + 3]
                      ay2 = anc_s[:, 4 * t + 3:4 * t + 4]
                      if j < ydn:
                          t2y = wp.tile([P, G2], F32, tag="t2y", name="t2y")
                          nc.vector.tensor_scalar(t2y[:], gy1b[:], ay1, 1.0,
                                                  op0=ALU.max,
                                                  op1=ALU.subtract)
                          nc.vector.scalar_tensor_tensor(
                              ih_st[:, j, :], gy2b[:], ay2, t2y[:],
                              op0=ALU.min, op1=ALU.subtract)
                      else:
                          # y axis on ACT: two hinge terms
                          nc.scalar.activation(ry1_st[:, j, :], gy1b[:],
                                               ACTF.Relu,
                                               bias=nay1_all[:, t:t + 1])
                          nc.scalar.activation(ry2_st[:, j, :], gy2b[:],
                                               ACTF.Relu, bias=ay2,
                                               scale=-1.0)
                      # x axis on DVE
                      t2x = wp.tile([P, G2], F32, tag="t2x", name="t2x")
                      nc.vector.tensor_scalar(t2x[:], gx1b[:], ax1, 1.0,
                                              op0=ALU.max, op1=ALU.subtract)
                      nc.vector.scalar_tensor_tensor(iw0_st[:, j, :], gx2b[:],
                                                     ax2, t2x[:], op0=ALU.min,
                                                     op1=ALU.subtract)
                  bsl = (slice(None), slice(0, TB), slice(None))
                  asl = (slice(None), slice(ydn, TB), slice(None))
                  csl = (slice(None), slice(0, ydn), slice(None))

                  def _eng(key):
                      return nc.gpsimd if kn[key] == "P" else nc.vector
                  # batched: sy = ry1 + ry2 (ACT-y slices), iw clamp in place
                  if ydn < TB:
                      _eng("sy_eng").tensor_tensor(sy_st[asl], ry1_st[asl],
                                                   ry2_st[asl], op=ALU.add)
                  _eng("iw_eng").tensor_scalar(iw0_st[bsl], iw0_st[bsl], 0.0,
                                               None, op0=ALU.max)
                  for j in range(ydn, TB):
                      t = gt0 + j
                      # ih = relu(eh - sy) per tile on ACT
                      nc.scalar.activation(ih_st[:, j, :], sy_st[:, j, :],
                                           ACTF.Relu,
                                           bias=eh_all[:, t:t + 1],
                                           scale=-1.0)
                  if ydn:
                      # clamp ih for the DVE-y slices
                      _eng("ih_eng").tensor_scalar(ih_st[csl], ih_st[csl],
                                                   0.0, None, op0=ALU.max)
                  # batched: inter = iw * ih
                  _eng("inter_eng").tensor_tensor(inter_st[bsl], iw0_st[bsl],
                                                  ih_st[bsl], op=ALU.mult)
                  for j in range(TB):
                      t = gt0 + j
                      aa = area_a[:, t:t + 1]
                      nc.vector.scalar_tensor_tensor(union_st[:, j, :],
                                                     area_gb[:], aa,
                                                     inter_st[:, j, :],
                                                     op0=ALU.add,
                                                     op1=ALU.subtract)
                  # batched: urec = 1/union (DVE, in place), iou
                  nc.vector.reciprocal(union_st[bsl], union_st[bsl])
                  _eng("iou_eng").tensor_tensor(iou_st[bsl], inter_st[bsl],
                                                union_st[bsl], op=ALU.mult)
                  return st

              def phase2(gt0, TB, st):
                  bsl = (slice(None), slice(0, TB), slice(None))
                  iou_st, mrev_st = st["iou_st"], st["mrev_st"]
                  # batched max-iou over the group
                  nc.vector.tensor_reduce(minv_all[:, gt0:gt0 + TB],
                                          iou_st[bsl], axis=AX.X, op=ALU.max)
                  for j in range(TB):
                      t = gt0 + j
                      nc.vector.scalar_tensor_tensor(
                          mrev_st[:, j, :], iou_st[:, j, :],
                          minv_all[:, t:t + 1], revp_b[:],
                          op0=ALU.is_equal, op1=ALU.mult)
                  nc.vector.tensor_reduce(rmax_all[:, gt0:gt0 + TB],
                                          mrev_st[bsl], axis=AX.X, op=ALU.max)
                  # onebias = 1 - rmax (tiny, Pool)
                  nc.gpsimd.tensor_scalar(onebias_all[:, gt0:gt0 + TB],
                                          rmax_all[:, gt0:gt0 + TB],
                                          -1.0, 1.0, op0=ALU.mult,
                                          op1=ALU.add)
                  ohT4 = wp.tile([G2, TBMAX * P], F32, tag="ohT4",
                                 name="ohT4", bufs=2)
                  gps4 = pp.tile([P, TBMAX * 5], F32, tag="gps4", name="gps4")
                  for h0 in range(0, TB, 4):
                      hn = min(4, TB - h0)
                      psT4 = pp.tile([G2, 4 * P], F32, tag="psT4",
                                     name="psT4")
                      for j in range(h0, h0 + hn):
                          t = gt0 + j
                          onehot = wp.tile([P, G2], F32, tag="onehot",
                                           name="onehot")
                          if kn["onehot"] == 0 or (kn["onehot"] == 2
                                                   and t % 2 == 0):
                              nc.vector.tensor_scalar(onehot[:],
                                                      mrev_st[:, j, :],
                                                      rmax_all[:, t:t + 1],
                                                      None, op0=ALU.is_equal)
                          else:
                              # exact: mrev integer-valued, rmax its max
                              nc.scalar.activation(
                                  onehot[:], mrev_st[:, j, :], ACTF.Relu,
                                  bias=onebias_all[:, t:t + 1])
                          nc.tensor.transpose(psT4[:, (j - h0) * P:
                                                   (j - h0 + 1) * P],
                                              onehot[:], identity[:])
                      oc = kn["ohTcopy"]
                      if oc == "D" or (oc == "X" and (gt0 // TBMAX) % 2 == 0):
                          nc.vector.tensor_copy(ohT4[:, h0 * P:(h0 + hn) * P],
                                                psT4[:, 0:hn * P])
                      else:
                          nc.scalar.copy(ohT4[:, h0 * P:(h0 + hn) * P],
                                         psT4[:, 0:hn * P])
                  for j in range(TB):
                      nc.tensor.matmul(gps4[:, j * 5:(j + 1) * 5],
                                       ohT4[:, j * P:(j + 1) * P],
                                       gtv[:, 0:5], start=True, stop=True)
                  nc.scalar.copy(gath[:, 5 * gt0:5 * (gt0 + TB)],
                                 gps4[:, 0:TB * 5])

              # software-pipelined emission: phase1(g+1) before phase2(g)
              pend = []
              for (gt0, TB) in groups:
                  pend.append((gt0, TB, phase1(gt0, TB)))
                  if len(pend) > kn.get("skew", 1):
                      g0, tb0, st0 = pend.pop(0)
                      phase2(g0, tb0, st0)
              for (g0, tb0, st0) in pend:
                  phase2(g0, tb0, st0)
              # ------------------------------------------------ epilogue
              gv = gath[:].rearrange("p (t c) -> p t c", c=5)
              ob = outbuf[:].rearrange("p (t c) -> p t c", c=5)
              tmp1 = cp.tile([P, NT], F32)
              tmp2 = cp.tile([P, NT], F32)
              lm1 = cp.tile([P, NT], F32)
              lm2 = cp.tile([P, NT], F32)
              for c_ in range(NCH):
                  cs = slice(c_ * CH, NT if c_ == NCH - 1 else (c_ + 1) * CH)
                  # dx, dy
                  nc.vector.tensor_sub(tmp1[:, cs], gv[:, cs, 0],
                                       ecx_all[:, cs])
                  nc.vector.tensor_tensor(ob[:, cs, 1], tmp1[:, cs],
                                          iew_all[:, cs], op=ALU.mult)
                  nc.vector.tensor_sub(tmp2[:, cs], gv[:, cs, 1],
                                       ecy_all[:, cs])
                  nc.vector.tensor_tensor(ob[:, cs, 2], tmp2[:, cs],
                                          ieh_all[:, cs], op=ALU.mult)
                  # dw, dh
                  nc.vector.tensor_sub(ob[:, cs, 3], gv[:, cs, 2],
                                       lew_all[:, cs])
                  nc.vector.tensor_sub(ob[:, cs, 4], gv[:, cs, 3],
                                       leh_all[:, cs])
                  # labels: pos*(cls+1) + neg - 1
                  nc.vector.scalar_tensor_tensor(lm1[:, cs], minv_all[:, cs],
                                                 0.5, gv[:, cs, 4],
                                                 op0=ALU.is_ge, op1=ALU.mult)
                  nc.vector.scalar_tensor_tensor(lm2[:, cs], minv_all[:, cs],
                                                 0.4, lm1[:, cs],
                                                 op0=ALU.is_lt, op1=ALU.add)
                  nc.vector.tensor_scalar_add(ob[:, cs, 0], lm2[:, cs], -1.0)
              nc.sync.dma_start(out_ext.ap(), outbuf[:])

    nc.compile()
    return nc


_NC = {}


def _get_nc(reps: int = 1):
    if reps not in _NC:
        _NC[reps] = build_nc(reps)
    return _NC[reps]


def make_in_maps(gt_boxes):
    anchors = _shift(FW, FH, STRIDE, _generate_anchors(ANCHOR_SIZE))
    gt = np.asarray(gt_boxes, dtype=np.float32)[0]          # [G, 5]
    in_maps = []
    for c in range(NCORES):
        shard = anchors[c * NA:(c + 1) * NA]
        pad = np.zeros((NAP - NA, 4), dtype=np.float32)
        sh = np.concatenate([shard, pad], axis=0)           # [NAP, 4]
        anc = np.ascontiguousarray(
            sh.reshape(NT, P, 4).transpose(1, 0, 2).reshape(P, NT * 4))
        lo = np.float32(shard[:, 1].min() - 1.0)
        hi = np.float32(shard[:, 3].max() + 1.0)
        nkeep = 1 + int(np.sum((np.minimum(gt[1:, 3], hi)
                                - np.maximum(gt[1:, 1], lo) + 1.0) > 0))
        assert nkeep <= G2, f"core {c}: {nkeep} relevant gt boxes > G2={G2}"
        band = np.tile(np.array([[lo, hi]], dtype=np.float32), (G, 1))
        in_maps.append({"anc": anc, "gt": gt, "band": band})
    return anchors, in_maps


def kernel(gt_boxes, fw, fh):
    assert int(fw) == FW and int(fh) == FH
    anchors, in_maps = make_in_maps(gt_boxes)
    nc = _get_nc()
    res = run_bass_kernel_spmd(nc, in_maps, core_ids=list(range(NCORES)))
    parts = []
    for c in range(NCORES):
        o = res.results[c]["out"]                           # [P, NT*5]
        o = o.reshape(P, NT, 5).transpose(1, 0, 2).reshape(NAP, 5)[:NA]
        parts.append(o)
    full = np.concatenate(parts, axis=0)                    # [N, 5]
    labels = np.ascontiguousarray(full[:, 0])[None]
    targets = np.ascontiguousarray(full[:, 1:5])[None]
    return labels, targets, anchors[None]


if __name__ == "__main__":
    gt = np.random.rand(1, G, 5).astype(np.float32)
    out = kernel(gt_boxes=gt, fw=FW, fh=FH)
    print([o.shape for o in out])


# revision 24
# speedup vs baseline: 1.1469x; 1.0663x over previous
"""AnchorTarget kernel for 8 TRN2 NeuronCores (Bass/Tile).

Strategy: shard the anchor dimension N=273600 across 8 cores (34200 each,
padded to 268*128=34304); each core's anchors cover a contiguous 19-row
y-band of the feature grid. gt_boxes (100 boxes) are replicated, and each
core COMPACTS on-device the boxes that can overlap its band (box 0 is
always kept first so all-zero-IoU rows resolve to gt[0] like jnp.argmax)
into G2=48 slots via a prefix-sum rank (triangular-ones PE matmul) and a
one-hot selection matrix. The hot loop then computes IoU, first-index
argmax (reverse-iota trick) and the matched-gt gather (PE transpose +
matmul against a per-slot table [gcx, gcy, ln gw, ln gh, cls+1]) on
[128, 48] tiles, with per-op engine assignment balanced across
DVE/ACT/Pool and group-batched reductions.
"""

import sys

for _p in ("/opt/trn_rl_repo",):
    if _p not in sys.path:
        sys.path.insert(0, _p)

import numpy as np

import concourse.bass as bass
import concourse.bacc as bacc
import concourse.mybir as mybir
from concourse import tile, masks
from concourse.bass_utils import run_bass_kernel_spmd

F32 = mybir.dt.float32
ALU = mybir.AluOpType
ACTF = mybir.ActivationFunctionType
AX = mybir.AxisListType

STRIDE = 8
ANCHOR_SIZE = 32
FW, FH = 200, 152
G = 100
G2 = 40                    # compacted gt slots per core
A = 9
N = FW * FH * A            # 273600
NCORES = 8
NA = N // NCORES           # 34200 anchors per core
P = 128
NT = (NA + P - 1) // P     # 268 tiles per core
NAP = NT * P               # 34304 padded per core

RATIOS = np.array([0.5, 1.0, 2.0])
SCALES = np.array([2.0 ** 0, 2.0 ** (1.0 / 3.0), 2.0 ** (2.0 / 3.0)])

KNOBS = {"onehot": 2, "y_dve_n": 3, "wbufs": 8, "sbufs": 6, "skew": 3,
         "sy_eng": "P", "iw_eng": "P", "ih_eng": "P", "inter_eng": "P",
         "iou_eng": "P", "tbmax": 8, "ohTcopy": "A",
         "asum_eng": "P", "union_eng": "P"}


# ---------------------------------------------------------------- host anchors
def _whctrs(a):
    w = a[2] - a[0] + 1.0
    h = a[3] - a[1] + 1.0
    xc = a[0] + 0.5 * (w - 1.0)
    yc = a[1] + 0.5 * (h - 1.0)
    return w, h, xc, yc


def _mkanchors(ws, hs, xc, yc):
    ws = ws[:, None]
    hs = hs[:, None]
    return np.hstack([xc - 0.5 * (ws - 1.0), yc - 0.5 * (hs - 1.0),
                      xc + 0.5 * (ws - 1.0), yc + 0.5 * (hs - 1.0)])


def _generate_anchors(base_size):
    base = np.array([0.0, 0.0, base_size - 1.0, base_size - 1.0])
    w, h, xc, yc = _whctrs(base)
    size_ratios = (w * h) / RATIOS
    ws = np.round(np.sqrt(size_ratios))
    hs = np.round(ws * RATIOS)
    ratio_anchors = _mkanchors(ws, hs, xc, yc)
    out = []
    for i in range(ratio_anchors.shape[0]):
        w, h, xc, yc = _whctrs(ratio_anchors[i])
        out.append(_mkanchors(w * SCALES, h * SCALES, xc, yc))
    return np.vstack(out).astype(np.float32)


def _shift(fw, fh, stride, anchors):
    sx = np.arange(fw) * stride
    sy = np.arange(fh) * stride
    sx, sy = np.meshgrid(sx, sy)
    shifts = np.stack([sx.ravel(), sy.ravel(), sx.ravel(), sy.ravel()], axis=1)
    alla = anchors[None, :, :] + shifts[:, None, :].astype(np.float32)
    return alla.reshape(-1, 4).astype(np.float32)


# ---------------------------------------------------------------- device graph
def build_nc(reps: int = 1):
    kn = KNOBS
    nc = bacc.Bacc("TRN2", target_bir_lowering=False, debug=False,
                   num_devices=NCORES)

    anc_ext = nc.dram_tensor("anc", [P, NT * 4], F32, kind="ExternalInput")
    gt_ext = nc.dram_tensor("gt", [G, 5], F32, kind="ExternalInput")
    band_ext = nc.dram_tensor("band", [G, 2], F32, kind="ExternalInput")
    out_ext = nc.dram_tensor("out", [P, NT * 5], F32, kind="ExternalOutput")

    with tile.TileContext(nc) as tc:
        from contextlib import ExitStack
        ctx = ExitStack()
        with ctx:
            cp = ctx.enter_context(tc.tile_pool(name="const", bufs=1))
            wp = ctx.enter_context(
                tc.tile_pool(name="work", bufs=kn["wbufs"]))
            pp = ctx.enter_context(tc.tile_pool(name="psum", bufs=3,
                                                space="PSUM"))

            # persistent buffers
            anc_s = cp.tile([P, NT * 4], F32)
            outbuf = cp.tile([P, NT * 5], F32)
            gath = cp.tile([P, NT * 5], F32)
            minv_all = cp.tile([P, NT], F32)
            rmax_all = cp.tile([P, NT], F32)
            onebias_all = cp.tile([P, NT], F32)
            gt_s = cp.tile([G, 5], F32)
            band_s = cp.tile([G, 2], F32)
            identity = cp.tile([P, P], F32)
            ltri = cp.tile([G, G], F32)
            ones = cp.tile([1, P], F32)
            slotids = cp.tile([P, G2], F32)
            gtv = cp.tile([G2, 5], F32)
            gt_c = cp.tile([G2, 5], F32)
            rows4 = cp.tile([1, 4 * G2], F32)
            dummyv = cp.tile([1, 4], F32)
            keep = cp.tile([G, 1], F32)
            rank = cp.tile([G, 1], F32)
            rank_m = cp.tile([G, 1], F32)
            notf = cp.tile([1, G2], F32)
            st_sel = cp.tile([G, G2], F32)
            gx1b = cp.tile([P, G2], F32)
            gy1b = cp.tile([P, G2], F32)
            gx2b = cp.tile([P, G2], F32)
            gy2b = cp.tile([P, G2], F32)
            area_gb = cp.tile([P, G2], F32)
            revp_b = cp.tile([P, G2], F32)
            # per-anchor batched quantities
            ew_all = cp.tile([P, NT], F32)
            eh_all = cp.tile([P, NT], F32)
            area_a = cp.tile([P, NT], F32)
            ecx_all = cp.tile([P, NT], F32)
            ecy_all = cp.tile([P, NT], F32)
            iew_all = cp.tile([P, NT], F32)
            ieh_all = cp.tile([P, NT], F32)
            lew_all = cp.tile([P, NT], F32)
            leh_all = cp.tile([P, NT], F32)
            nay1_all = cp.tile([P, NT], F32)

            # input DMAs
            nc.sync.dma_start(gt_s[:], gt_ext.ap())
            nc.sync.dma_start(band_s[:], band_ext.ap())

            # constants: identity, lower-tri ones, ones row, slot iota
            masks.make_identity(nc, identity[:])
            nc.gpsimd.memset(ltri[:], 0.0)
            nc.gpsimd.affine_select(
                out=ltri[:], in_=ltri[:], compare_op=ALU.is_gt, fill=1.0,
                base=0, pattern=[[-1, G]], channel_multiplier=1)
            nc.gpsimd.memset(ones[:], 1.0)
            slot_i = cp.tile([P, G2], mybir.dt.int32)
            nc.gpsimd.iota(slot_i[:], pattern=[[1, G2]], base=0,
                           channel_multiplier=0)
            nc.vector.tensor_copy(slotids[:], slot_i[:])
            revp_i = cp.tile([P, G2], mybir.dt.int32)
            nc.gpsimd.iota(revp_i[:], pattern=[[-1, G2]], base=G2,
                           channel_multiplier=0)
            nc.vector.tensor_copy(revp_b[:], revp_i[:])
            nc.gpsimd.memset(dummyv[:, 0:2], -100000.0)
            nc.gpsimd.memset(dummyv[:, 2:4], -99999.0)

            # ---- gt band compaction ----
            gy1, gy2 = gt_s[:, 1:2], gt_s[:, 3:4]
            blo, bhi = band_s[:, 0:1], band_s[:, 1:2]
            m1 = cp.tile([G, 1], F32)
            m2 = cp.tile([G, 1], F32)
            nc.vector.tensor_tensor(m1[:], gy2, bhi, op=ALU.min)
            nc.vector.tensor_tensor(m2[:], gy1, blo, op=ALU.max)
            k0 = cp.tile([G, 1], F32)
            nc.vector.scalar_tensor_tensor(k0[:], m1[:], 1.0, m2[:],
                                           op0=ALU.add, op1=ALU.subtract)
            nc.vector.tensor_scalar(keep[:], k0[:], 0.0, None, op0=ALU.is_gt)
            nc.gpsimd.memset(keep[0:1, :], 1.0)
            # inclusive prefix sum of keep via lower-triangular ones
            ps_rank = pp.tile([G, 1], F32, tag="setup", bufs=2, name="ps_rank")
            nc.tensor.matmul(ps_rank[:], ltri[:], keep[:], start=True,
                             stop=True)
            nc.scalar.copy(rank[:], ps_rank[:])
            # slot = rank-1 for kept, >=999 for dropped
            a999 = cp.tile([G, 1], F32)
            nc.vector.tensor_scalar_add(a999[:], rank[:], 999.0)
            nc.vector.scalar_tensor_tensor(rank_m[:], keep[:], -1000.0,
                                           a999[:], op0=ALU.mult, op1=ALU.add)
            # selection matrix [g, slot] and unfilled-slot row
            nc.vector.tensor_scalar(st_sel[:], slotids[0:G, :], rank_m[:],
                                    None, op0=ALU.is_equal)
            ones100 = cp.tile([G, 1], F32)
            nc.gpsimd.memset(ones100[:], 1.0)
            ps_cnt = pp.tile([1, 1], F32, tag="setup", bufs=2, name="ps_cnt")
            nc.tensor.matmul(ps_cnt[:], keep[:], ones100[:], start=True,
                             stop=True)
            cnt = cp.tile([1, 1], F32)
            nc.scalar.copy(cnt[:], ps_cnt[:])
            nc.vector.tensor_scalar(notf[:], slotids[0:1, :], cnt[:, 0:1],
                                    None, op0=ALU.is_ge)
            # compacted gt rows [G2, 5] (+dummy box into unfilled slots)
            ps_gtc = pp.tile([G2, 5], F32, tag="setup", bufs=2, name="ps_gtc")
            nc.tensor.matmul(ps_gtc[:], st_sel[:], gt_s[:], start=True,
                             stop=False)
            dummy5 = cp.tile([1, 5], F32)
            nc.vector.tensor_copy(dummy5[:, 0:4], dummyv[:])
            nc.gpsimd.memset(dummy5[:, 4:5], 0.0)
            nc.tensor.matmul(ps_gtc[:], notf[:], dummy5[:], start=False,
                             stop=True)
            nc.scalar.copy(gt_c[:], ps_gtc[:])
            # compacted coord rows [1, 4*G2] then broadcast to [128, G2]
            ps_rows = pp.tile([1, 4 * G2], F32, tag="setup", bufs=2,
                              name="ps_rows")
            for r in range(4):
                nc.tensor.matmul(ps_rows[:, r * G2:(r + 1) * G2],
                                 gt_s[:, r:r + 1], st_sel[:], start=True,
                                 stop=False)
                nc.tensor.matmul(ps_rows[:, r * G2:(r + 1) * G2],
                                 dummyv[:, r:r + 1], notf[:], start=False,
                                 stop=True)
            nc.scalar.copy(rows4[:], ps_rows[:])
            for r, dst in enumerate([gx1b, gy1b, gx2b, gy2b]):
                ps = pp.tile([P, G2], F32, tag="setup", bufs=2, name="psbc")
                nc.tensor.matmul(ps[:], ones[:],
                                 rows4[:, r * G2:(r + 1) * G2],
                                 start=True, stop=True)
                nc.scalar.copy(dst[:], ps[:])

            # area_g broadcast: (gx2-gx1+1)*(gy2-gy1+1)
            wg = cp.tile([P, G2], F32)
            hg = cp.tile([P, G2], F32)
            nc.vector.scalar_tensor_tensor(wg[:], gx1b[:], -1.0, gx2b[:],
                                           op0=ALU.mult, op1=ALU.add)
            nc.vector.tensor_scalar_add(wg[:], wg[:], 1.0)
            nc.vector.scalar_tensor_tensor(hg[:], gy1b[:], -1.0, gy2b[:],
                                           op0=ALU.mult, op1=ALU.add)
            nc.vector.tensor_scalar_add(hg[:], hg[:], 1.0)
            nc.vector.tensor_tensor(area_gb[:], wg[:], hg[:], op=ALU.mult)

            # per-slot gather table: [gcx, gcy, ln gw, ln gh, cls+1]
            x1, y1 = gt_c[:, 0:1], gt_c[:, 1:2]
            x2, y2 = gt_c[:, 2:3], gt_c[:, 3:4]
            cls = gt_c[:, 4:5]
            gw = cp.tile([G2, 1], F32)
            gh = cp.tile([G2, 1], F32)
            nc.vector.scalar_tensor_tensor(gw[:], x1, -1.0, x2,
                                           op0=ALU.mult, op1=ALU.add)
            nc.vector.tensor_scalar_add(gw[:], gw[:], 1.0)
            nc.vector.scalar_tensor_tensor(gh[:], y1, -1.0, y2,
                                           op0=ALU.mult, op1=ALU.add)
            nc.vector.tensor_scalar_add(gh[:], gh[:], 1.0)
            nc.vector.scalar_tensor_tensor(gtv[:, 0:1], gw[:], 0.5, x1,
                                           op0=ALU.mult, op1=ALU.add)
            nc.vector.scalar_tensor_tensor(gtv[:, 1:2], gh[:], 0.5, y1,
                                           op0=ALU.mult, op1=ALU.add)
            nc.scalar.activation(gtv[:, 2:3], gw[:], ACTF.Ln)
            nc.scalar.activation(gtv[:, 3:4], gh[:], ACTF.Ln)
            nc.vector.tensor_scalar_add(gtv[:, 4:5], cls, 1.0)

            # ---- repeated body (reps>1 used only for slope timing) ----
            for _rep in range(reps):
              nc.sync.dma_start(anc_s[:], anc_ext.ap())
              av = anc_s[:].rearrange("p (t c) -> p t c", c=4)
              NCH = 4
              CH = NT // NCH
              for c_ in range(NCH):
                  cs = slice(c_ * CH, NT if c_ == NCH - 1 else (c_ + 1) * CH)
                  ax1v, ay1v = av[:, cs, 0], av[:, cs, 1]
                  ax2v, ay2v = av[:, cs, 2], av[:, cs, 3]
                  nc.vector.scalar_tensor_tensor(ew_all[:, cs], ax1v, -1.0,
                                                 ax2v, op0=ALU.mult,
                                                 op1=ALU.add)
                  nc.vector.tensor_scalar_add(ew_all[:, cs], ew_all[:, cs],
                                              1.0)
                  nc.vector.scalar_tensor_tensor(eh_all[:, cs], ay1v, -1.0,
                                                 ay2v, op0=ALU.mult,
                                                 op1=ALU.add)
                  nc.vector.tensor_scalar_add(eh_all[:, cs], eh_all[:, cs],
                                              1.0)
                  nc.gpsimd.tensor_tensor(area_a[:, cs], ew_all[:, cs],
                                          eh_all[:, cs], op=ALU.mult)
                  nc.vector.scalar_tensor_tensor(ecx_all[:, cs],
                                                 ew_all[:, cs], 0.5, ax1v,
                                                 op0=ALU.mult, op1=ALU.add)
                  nc.vector.scalar_tensor_tensor(ecy_all[:, cs],
                                                 eh_all[:, cs], 0.5, ay1v,
                                                 op0=ALU.mult, op1=ALU.add)
                  nc.vector.tensor_scalar_mul(nay1_all[:, cs], ay1v, -1.0)
                  nc.vector.reciprocal(iew_all[:, cs], ew_all[:, cs])
                  nc.vector.reciprocal(ieh_all[:, cs], eh_all[:, cs])
                  nc.scalar.activation(lew_all[:, cs], ew_all[:, cs], ACTF.Ln)
                  nc.scalar.activation(leh_all[:, cs], eh_all[:, cs], ACTF.Ln)

              # ------------------------------------------------ main loop
              TBMAX = kn["tbmax"]
              groups = []
              t0 = 0
              while t0 < NT:
                  tb = min(TBMAX, NT - t0)
                  groups.append((t0, tb))
                  t0 += tb
              def phase1(gt0, TB):
                  st = {}
                  st["ry1_st"] = wp.tile([P, TBMAX, G2], F32, tag="ry1_st",
                                         name="ry1_st", bufs=kn["sbufs"])
                  st["ry2_st"] = wp.tile([P, TBMAX, G2], F32, tag="ry2_st",
                                         name="ry2_st", bufs=kn["sbufs"])
                  st["iw0_st"] = wp.tile([P, TBMAX, G2], F32, tag="iw0_st",
                                         name="iw0_st", bufs=kn["sbufs"])
                  st["sy_st"] = wp.tile([P, TBMAX, G2], F32, tag="sy_st",
                                        name="sy_st", bufs=kn["sbufs"])
                  st["ih_st"] = wp.tile([P, TBMAX, G2], F32, tag="ih_st",
                                        name="ih_st", bufs=kn["sbufs"])
                  st["inter_st"] = wp.tile([P, TBMAX, G2], F32,
                                           tag="inter_st", name="inter_st",
                                           bufs=kn["sbufs"])
                  st["union_st"] = wp.tile([P, TBMAX, G2], F32,
                                           tag="union_st", name="union_st",
                                           bufs=kn["sbufs"])
                  st["iou_st"] = wp.tile([P, TBMAX, G2], F32, tag="iou_st",
                                         name="iou_st", bufs=kn["sbufs"])
                  st["mrev_st"] = wp.tile([P, TBMAX, G2], F32, tag="mrev_st",
                                          name="mrev_st", bufs=kn["sbufs"])
                  ry1_st, ry2_st = st["ry1_st"], st["ry2_st"]
                  iw0_st, sy_st, ih_st = st["iw0_st"], st["sy_st"], st["ih_st"]
                  inter_st, union_st = st["inter_st"], st["union_st"]
                  iou_st = st["iou_st"]
                  ydn = min(kn["y_dve_n"], TB)
                  st["ydn"] = ydn
                  for j in range(TB):
                      t = gt0 + j
                      ax1 = anc_s[:, 4 * t + 0:4 * t + 1]
                      ay1 = anc_s[:, 4 * t + 1:4 * t + 2]
                      ax2 = anc_s[:, 4 * t + 2:4 * t + 3]
                      ay2 = anc_s[:, 4 * t + 3:4 * t + 4]
                      if j < ydn:
                          t2y = wp.tile([P, G2], F32, tag="t2y", name="t2y")
                          nc.vector.tensor_scalar(t2y[:], gy1b[:], ay1, 1.0,
                                                  op0=ALU.max,
                                                  op1=ALU.subtract)
                          nc.vector.scalar_tensor_tensor(
                              ih_st[:, j, :], gy2b[:], ay2, t2y[:],
                              op0=ALU.min, op1=ALU.subtract)
                      else:
                          # y axis on ACT: two hinge terms
                          nc.scalar.activation(ry1_st[:, j, :], gy1b[:],
                                               ACTF.Relu,
                                               bias=nay1_all[:, t:t + 1])
                          nc.scalar.activation(ry2_st[:, j, :], gy2b[:],
                                               ACTF.Relu, bias=ay2,
                                               scale=-1.0)
                      # x axis on DVE
                      t2x = wp.tile([P, G2], F32, tag="t2x", name="t2x")
                      nc.vector.tensor_scalar(t2x[:], gx1b[:], ax1, 1.0,
                                              op0=ALU.max, op1=ALU.subtract)
                      nc.vector.scalar_tensor_tensor(iw0_st[:, j, :], gx2b[:],
                                                     ax2, t2x[:], op0=ALU.min,
                                                     op1=ALU.subtract)
                  bsl = (slice(None), slice(0, TB), slice(None))
                  asl = (slice(None), slice(ydn, TB), slice(None))
                  csl = (slice(None), slice(0, ydn), slice(None))

                  def _eng(key):
                      return nc.gpsimd if kn[key] == "P" else nc.vector
                  # batched: sy = ry1 + ry2 (ACT-y slices), iw clamp in place
                  if ydn < TB:
                      _eng("sy_eng").tensor_tensor(sy_st[asl], ry1_st[asl],
                                                   ry2_st[asl], op=ALU.add)
                  _eng("iw_eng").tensor_scalar(iw0_st[bsl], iw0_st[bsl], 0.0,
                                               None, op0=ALU.max)
                  for j in range(ydn, TB):
                      t = gt0 + j
                      # ih = relu(eh - sy) per tile on ACT
                      nc.scalar.activation(ih_st[:, j, :], sy_st[:, j, :],
                                           ACTF.Relu,
                                           bias=eh_all[:, t:t + 1],
                                           scale=-1.0)
                  if ydn:
                      # clamp ih for the DVE-y slices
                      _eng("ih_eng").tensor_scalar(ih_st[csl], ih_st[csl],
                                                   0.0, None, op0=ALU.max)
                  # batched: inter = iw * ih
                  _eng("inter_eng").tensor_tensor(inter_st[bsl], iw0_st[bsl],
                                                  ih_st[bsl], op=ALU.mult)
                  aa3 = (area_a[:, gt0:gt0 + TB]
                         .rearrange("p (t o) -> p t o", o=1)
                         .broadcast_to([P, TB, G2]))
                  ag3 = (area_gb[:]
                         .rearrange("p (o g) -> p o g", o=1)
                         .broadcast_to([P, TB, G2]))
                  _eng("asum_eng").tensor_tensor(union_st[bsl], aa3, ag3,
                                                 op=ALU.add)
                  _eng("union_eng").tensor_tensor(union_st[bsl],
                                                  union_st[bsl],
                                                  inter_st[bsl],
                                                  op=ALU.subtract)
                  # batched: urec = 1/union (DVE, in place), iou
                  nc.vector.reciprocal(union_st[bsl], union_st[bsl])
                  _eng("iou_eng").tensor_tensor(iou_st[bsl], inter_st[bsl],
                                                union_st[bsl], op=ALU.mult)
                  return st

              def phase2(gt0, TB, st):
                  bsl = (slice(None), slice(0, TB), slice(None))
                  iou_st, mrev_st = st["iou_st"], st["mrev_st"]
                  # batched max-iou over the group
                  nc.vector.tensor_reduce(minv_all[:, gt0:gt0 + TB],
                                          iou_st[bsl], axis=AX.X, op=ALU.max)
                  for j in range(TB):
                      t = gt0 + j
                      nc.vector.scalar_tensor_tensor(
                          mrev_st[:, j, :], iou_st[:, j, :],
                          minv_all[:, t:t + 1], revp_b[:],
                          op0=ALU.is_equal, op1=ALU.mult)
                  nc.vector.tensor_reduce(rmax_all[:, gt0:gt0 + TB],
                                          mrev_st[bsl], axis=AX.X, op=ALU.max)
                  # onebias = 1 - rmax (tiny, Pool)
                  nc.gpsimd.tensor_scalar(onebias_all[:, gt0:gt0 + TB],
                                          rmax_all[:, gt0:gt0 + TB],
                                          -1.0, 1.0, op0=ALU.mult,
                                          op1=ALU.add)
                  ohT4 = wp.tile([G2, TBMAX * P], F32, tag="ohT4",
                                 name="ohT4", bufs=2)
                  gps4 = pp.tile([P, TBMAX * 5], F32, tag="gps4", name="gps4")
                  for h0 in range(0, TB, 4):
                      hn = min(4, TB - h0)
                      psT4 = pp.tile([G2, 4 * P], F32, tag="psT4",
                                     name="psT4")
                      for j in range(h0, h0 + hn):
                          t = gt0 + j
                          onehot = wp.tile([P, G2], F32, tag="onehot",
                                           name="onehot")
                          if kn["onehot"] == 0 or (kn["onehot"] == 2
                                                   and t % 2 == 0):
                              nc.vector.tensor_scalar(onehot[:],
                                                      mrev_st[:, j, :],
                                                      rmax_all[:, t:t + 1],
                                                      None, op0=ALU.is_equal)
                          else:
                              # exact: mrev integer-valued, rmax its max
                              nc.scalar.activation(
                                  onehot[:], mrev_st[:, j, :], ACTF.Relu,
                                  bias=onebias_all[:, t:t + 1])
                          nc.tensor.transpose(psT4[:, (j - h0) * P:
                                                   (j - h0 + 1) * P],
                                              onehot[:], identity[:])
                      oc = kn["ohTcopy"]
                      if oc == "D" or (oc == "X" and (gt0 // TBMAX) % 2 == 0):
                          nc.vector.tensor_copy(ohT4[:, h0 * P:(h0 + hn) * P],
                                                psT4[:, 0:hn * P])
                      else:
                          nc.scalar.copy(ohT4[:, h0 * P:(h0 + hn) * P],
                                         psT4[:, 0:hn * P])
                  for j in range(TB):
                      nc.tensor.matmul(gps4[:, j * 5:(j + 1) * 5],
                                       ohT4[:, j * P:(j + 1) * P],
                                       gtv[:, 0:5], start=True, stop=True)
                  nc.scalar.copy(gath[:, 5 * gt0:5 * (gt0 + TB)],
                                 gps4[:, 0:TB * 5])

              # software-pipelined emission: phase1(g+1) before phase2(g)
              pend = []
              for (gt0, TB) in groups:
                  pend.append((gt0, TB, phase1(gt0, TB)))
                  if len(pend) > kn.get("skew", 1):
                      g0, tb0, st0 = pend.pop(0)
                      phase2(g0, tb0, st0)
              for (g0, tb0, st0) in pend:
                  phase2(g0, tb0, st0)
              # ------------------------------------------------ epilogue
              gv = gath[:].rearrange("p (t c) -> p t c", c=5)
              ob = outbuf[:].rearrange("p (t c) -> p t c", c=5)
              tmp1 = cp.tile([P, NT], F32)
              tmp2 = cp.tile([P, NT], F32)
              lm1 = cp.tile([P, NT], F32)
              lm2 = cp.tile([P, NT], F32)
              for c_ in range(NCH):
                  cs = slice(c_ * CH, NT if c_ == NCH - 1 else (c_ + 1) * CH)
                  # dx, dy
                  nc.vector.tensor_sub(tmp1[:, cs], gv[:, cs, 0],
                                       ecx_all[:, cs])
                  nc.vector.tensor_tensor(ob[:, cs, 1], tmp1[:, cs],
                                          iew_all[:, cs], op=ALU.mult)
                  nc.vector.tensor_sub(tmp2[:, cs], gv[:, cs, 1],
                                       ecy_all[:, cs])
                  nc.vector.tensor_tensor(ob[:, cs, 2], tmp2[:, cs],
                                          ieh_all[:, cs], op=ALU.mult)
                  # dw, dh
                  nc.vector.tensor_sub(ob[:, cs, 3], gv[:, cs, 2],
                                       lew_all[:, cs])
                  nc.vector.tensor_sub(ob[:, cs, 4], gv[:, cs, 3],
                                       leh_all[:, cs])
                  # labels: pos*(cls+1) + neg - 1
                  nc.vector.scalar_tensor_tensor(lm1[:, cs], minv_all[:, cs],
                                                 0.5, gv[:, cs, 4],
                                                 op0=ALU.is_ge, op1=ALU.mult)
                  nc.vector.scalar_tensor_tensor(lm2[:, cs], minv_all[:, cs],
                                                 0.4, lm1[:, cs],
                                                 op0=ALU.is_lt, op1=ALU.add)
                  nc.vector.tensor_scalar_add(ob[:, cs, 0], lm2[:, cs], -1.0)
              nc.sync.dma_start(out_ext.ap(), outbuf[:])

    nc.compile()
    return nc


_NC = {}


def _get_nc(reps: int = 1):
    if reps not in _NC:
        _NC[reps] = build_nc(reps)
    return _NC[reps]


def make_in_maps(gt_boxes):
    anchors = _shift(FW, FH, STRIDE, _generate_anchors(ANCHOR_SIZE))
    gt = np.asarray(gt_boxes, dtype=np.float32)[0]          # [G, 5]
    in_maps = []
    for c in range(NCORES):
        shard = anchors[c * NA:(c + 1) * NA]
        pad = np.zeros((NAP - NA, 4), dtype=np.float32)
        sh = np.concatenate([shard, pad], axis=0)           # [NAP, 4]
        anc = np.ascontiguousarray(
            sh.reshape(NT, P, 4).transpose(1, 0, 2).reshape(P, NT * 4))
        lo = np.float32(shard[:, 1].min() - 1.0)
        hi = np.float32(shard[:, 3].max() + 1.0)
        nkeep = 1 + int(np.sum((np.minimum(gt[1:, 3], hi)
                                - np.maximum(gt[1:, 1], lo) + 1.0) > 0))
        assert nkeep <= G2, f"core {c}: {nkeep} relevant gt boxes > G2={G2}"
        band = np.tile(np.array([[lo, hi]], dtype=np.float32), (G, 1))
        in_maps.append({"anc": anc, "gt": gt, "band": band})
    return anchors, in_maps


def kernel(gt_boxes, fw, fh):
    assert int(fw) == FW and int(fh) == FH
    anchors, in_maps = make_in_maps(gt_boxes)
    nc = _get_nc()
    res = run_bass_kernel_spmd(nc, in_maps, core_ids=list(range(NCORES)))
    parts = []
    for c in range(NCORES):
        o = res.results[c]["out"]                           # [P, NT*5]
        o = o.reshape(P, NT, 5).transpose(1, 0, 2).reshape(NAP, 5)[:NA]
        parts.append(o)
    full = np.concatenate(parts, axis=0)                    # [N, 5]
    labels = np.ascontiguousarray(full[:, 0])[None]
    targets = np.ascontiguousarray(full[:, 1:5])[None]
    return labels, targets, anchors[None]


if __name__ == "__main__":
    gt = np.random.rand(1, G, 5).astype(np.float32)
    out = kernel(gt_boxes=gt, fw=FW, fh=FH)
    print([o.shape for o in out])


# revision 27
# speedup vs baseline: 1.4030x; 1.2233x over previous
"""AnchorTarget kernel for 8 TRN2 NeuronCores (Bass/Tile).

Strategy: shard the anchor dimension N=273600 across 8 cores (34200 each,
padded to 268*128=34304); each core's anchors cover a contiguous 19-row
y-band of the feature grid. gt_boxes (100 boxes) are replicated, and each
core COMPACTS on-device the boxes that can overlap its band (box 0 is
always kept first so all-zero-IoU rows resolve to gt[0] like jnp.argmax)
into G2=48 slots via a prefix-sum rank (triangular-ones PE matmul) and a
one-hot selection matrix. The hot loop then computes IoU, first-index
argmax (reverse-iota trick) and the matched-gt gather (PE transpose +
matmul against a per-slot table [gcx, gcy, ln gw, ln gh, cls+1]) on
[128, 48] tiles, with per-op engine assignment balanced across
DVE/ACT/Pool and group-batched reductions.
"""

import sys

for _p in ("/opt/trn_rl_repo",):
    if _p not in sys.path:
        sys.path.insert(0, _p)

import numpy as np

import concourse.bass as bass
import concourse.bacc as bacc
import concourse.mybir as mybir
from concourse import tile, masks
from concourse.bass_utils import run_bass_kernel_spmd

F32 = mybir.dt.float32
ALU = mybir.AluOpType
ACTF = mybir.ActivationFunctionType
AX = mybir.AxisListType

STRIDE = 8
ANCHOR_SIZE = 32
FW, FH = 200, 152
G = 100
G2 = 40                    # compacted gt slots per core
A = 9
N = FW * FH * A            # 273600
NCORES = 8
NA = N // NCORES           # 34200 anchors per core
P = 128
NT = (NA + P - 1) // P     # 268 tiles per core
NAP = NT * P               # 34304 padded per core

RATIOS = np.array([0.5, 1.0, 2.0])
SCALES = np.array([2.0 ** 0, 2.0 ** (1.0 / 3.0), 2.0 ** (2.0 / 3.0)])

KNOBS = {"onehot": 2, "y_dve_n": 4, "wbufs": 8, "sbufs": 6, "skew": 3,
         "sy_eng": "P", "iw_eng": "P", "ih_eng": "P", "inter_eng": "P",
         "iou_eng": "P", "tbmax": 8, "ohTcopy": "A",
         "asum_eng": "D", "union_eng": "D",
         "m0_eng": "D", "mrevm_eng": "P", "oh_eng": "D"}


# ---------------------------------------------------------------- host anchors
def _whctrs(a):
    w = a[2] - a[0] + 1.0
    h = a[3] - a[1] + 1.0
    xc = a[0] + 0.5 * (w - 1.0)
    yc = a[1] + 0.5 * (h - 1.0)
    return w, h, xc, yc


def _mkanchors(ws, hs, xc, yc):
    ws = ws[:, None]
    hs = hs[:, None]
    return np.hstack([xc - 0.5 * (ws - 1.0), yc - 0.5 * (hs - 1.0),
                      xc + 0.5 * (ws - 1.0), yc + 0.5 * (hs - 1.0)])


def _generate_anchors(base_size):
    base = np.array([0.0, 0.0, base_size - 1.0, base_size - 1.0])
    w, h, xc, yc = _whctrs(base)
    size_ratios = (w * h) / RATIOS
    ws = np.round(np.sqrt(size_ratios))
    hs = np.round(ws * RATIOS)
    ratio_anchors = _mkanchors(ws, hs, xc, yc)
    out = []
    for i in range(ratio_anchors.shape[0]):
        w, h, xc, yc = _whctrs(ratio_anchors[i])
        out.append(_mkanchors(w * SCALES, h * SCALES, xc, yc))
    return np.vstack(out).astype(np.float32)


def _shift(fw, fh, stride, anchors):
    sx = np.arange(fw) * stride
    sy = np.arange(fh) * stride
    sx, sy = np.meshgrid(sx, sy)
    shifts = np.stack([sx.ravel(), sy.ravel(), sx.ravel(), sy.ravel()], axis=1)
    alla = anchors[None, :, :] + shifts[:, None, :].astype(np.float32)
    return alla.reshape(-1, 4).astype(np.float32)


# ---------------------------------------------------------------- device graph
def build_nc(reps: int = 1):
    kn = KNOBS
    nc = bacc.Bacc("TRN2", target_bir_lowering=False, debug=False,
                   num_devices=NCORES)

    anc_ext = nc.dram_tensor("anc", [P, NT * 4], F32, kind="ExternalInput")
    gt_ext = nc.dram_tensor("gt", [G, 5], F32, kind="ExternalInput")
    band_ext = nc.dram_tensor("band", [G, 2], F32, kind="ExternalInput")
    out_ext = nc.dram_tensor("out", [P, NT * 5], F32, kind="ExternalOutput")

    with tile.TileContext(nc) as tc:
        from contextlib import ExitStack
        ctx = ExitStack()
        with ctx:
            cp = ctx.enter_context(tc.tile_pool(name="const", bufs=1))
            wp = ctx.enter_context(
                tc.tile_pool(name="work", bufs=kn["wbufs"]))
            pp = ctx.enter_context(tc.tile_pool(name="psum", bufs=3,
                                                space="PSUM"))

            # persistent buffers
            anc_s = cp.tile([P, NT * 4], F32)
            outbuf = cp.tile([P, NT * 5], F32)
            gath = cp.tile([P, NT * 5], F32)
            minv_all = cp.tile([P, NT], F32)
            rmax_all = cp.tile([P, NT], F32)
            onebias_all = cp.tile([P, NT], F32)
            gt_s = cp.tile([G, 5], F32)
            band_s = cp.tile([G, 2], F32)
            identity = cp.tile([P, P], F32)
            ltri = cp.tile([G, G], F32)
            ones = cp.tile([1, P], F32)
            slotids = cp.tile([P, G2], F32)
            gtv = cp.tile([G2, 5], F32)
            gt_c = cp.tile([G2, 5], F32)
            rows4 = cp.tile([1, 4 * G2], F32)
            dummyv = cp.tile([1, 4], F32)
            keep = cp.tile([G, 1], F32)
            rank = cp.tile([G, 1], F32)
            rank_m = cp.tile([G, 1], F32)
            notf = cp.tile([1, G2], F32)
            st_sel = cp.tile([G, G2], F32)
            gx1b = cp.tile([P, G2], F32)
            gy1b = cp.tile([P, G2], F32)
            gx2b = cp.tile([P, G2], F32)
            gy2b = cp.tile([P, G2], F32)
            area_gb = cp.tile([P, G2], F32)
            revp_b = cp.tile([P, G2], F32)
            # per-anchor batched quantities
            ew_all = cp.tile([P, NT], F32)
            eh_all = cp.tile([P, NT], F32)
            area_a = cp.tile([P, NT], F32)
            ecx_all = cp.tile([P, NT], F32)
            ecy_all = cp.tile([P, NT], F32)
            iew_all = cp.tile([P, NT], F32)
            ieh_all = cp.tile([P, NT], F32)
            lew_all = cp.tile([P, NT], F32)
            leh_all = cp.tile([P, NT], F32)
            nay1_all = cp.tile([P, NT], F32)

            # input DMAs
            nc.sync.dma_start(gt_s[:], gt_ext.ap())
            nc.sync.dma_start(band_s[:], band_ext.ap())

            # constants: identity, lower-tri ones, ones row, slot iota
            masks.make_identity(nc, identity[:])
            nc.gpsimd.memset(ltri[:], 0.0)
            nc.gpsimd.affine_select(
                out=ltri[:], in_=ltri[:], compare_op=ALU.is_gt, fill=1.0,
                base=0, pattern=[[-1, G]], channel_multiplier=1)
            nc.gpsimd.memset(ones[:], 1.0)
            slot_i = cp.tile([P, G2], mybir.dt.int32)
            nc.gpsimd.iota(slot_i[:], pattern=[[1, G2]], base=0,
                           channel_multiplier=0)
            nc.vector.tensor_copy(slotids[:], slot_i[:])
            revp_i = cp.tile([P, G2], mybir.dt.int32)
            nc.gpsimd.iota(revp_i[:], pattern=[[-1, G2]], base=G2,
                           channel_multiplier=0)
            nc.vector.tensor_copy(revp_b[:], revp_i[:])
            nc.gpsimd.memset(dummyv[:, 0:2], -100000.0)
            nc.gpsimd.memset(dummyv[:, 2:4], -99999.0)

            # ---- gt band compaction ----
            gy1, gy2 = gt_s[:, 1:2], gt_s[:, 3:4]
            blo, bhi = band_s[:, 0:1], band_s[:, 1:2]
            m1 = cp.tile([G, 1], F32)
            m2 = cp.tile([G, 1], F32)
            nc.vector.tensor_tensor(m1[:], gy2, bhi, op=ALU.min)
            nc.vector.tensor_tensor(m2[:], gy1, blo, op=ALU.max)
            k0 = cp.tile([G, 1], F32)
            nc.vector.scalar_tensor_tensor(k0[:], m1[:], 1.0, m2[:],
                                           op0=ALU.add, op1=ALU.subtract)
            nc.vector.tensor_scalar(keep[:], k0[:], 0.0, None, op0=ALU.is_gt)
            nc.gpsimd.memset(keep[0:1, :], 1.0)
            # inclusive prefix sum of keep via lower-triangular ones
            ps_rank = pp.tile([G, 1], F32, tag="setup", bufs=2, name="ps_rank")
            nc.tensor.matmul(ps_rank[:], ltri[:], keep[:], start=True,
                             stop=True)
            nc.scalar.copy(rank[:], ps_rank[:])
            # slot = rank-1 for kept, >=999 for dropped
            a999 = cp.tile([G, 1], F32)
            nc.vector.tensor_scalar_add(a999[:], rank[:], 999.0)
            nc.vector.scalar_tensor_tensor(rank_m[:], keep[:], -1000.0,
                                           a999[:], op0=ALU.mult, op1=ALU.add)
            # selection matrix [g, slot] and unfilled-slot row
            nc.vector.tensor_scalar(st_sel[:], slotids[0:G, :], rank_m[:],
                                    None, op0=ALU.is_equal)
            ones100 = cp.tile([G, 1], F32)
            nc.gpsimd.memset(ones100[:], 1.0)
            ps_cnt = pp.tile([1, 1], F32, tag="setup", bufs=2, name="ps_cnt")
            nc.tensor.matmul(ps_cnt[:], keep[:], ones100[:], start=True,
                             stop=True)
            cnt = cp.tile([1, 1], F32)
            nc.scalar.copy(cnt[:], ps_cnt[:])
            nc.vector.tensor_scalar(notf[:], slotids[0:1, :], cnt[:, 0:1],
                                    None, op0=ALU.is_ge)
            # compacted gt rows [G2, 5] (+dummy box into unfilled slots)
            ps_gtc = pp.tile([G2, 5], F32, tag="setup", bufs=2, name="ps_gtc")
            nc.tensor.matmul(ps_gtc[:], st_sel[:], gt_s[:], start=True,
                             stop=False)
            dummy5 = cp.tile([1, 5], F32)
            nc.vector.tensor_copy(dummy5[:, 0:4], dummyv[:])
            nc.gpsimd.memset(dummy5[:, 4:5], 0.0)
            nc.tensor.matmul(ps_gtc[:], notf[:], dummy5[:], start=False,
                             stop=True)
            nc.scalar.copy(gt_c[:], ps_gtc[:])
            # compacted coord rows [1, 4*G2] then broadcast to [128, G2]
            ps_rows = pp.tile([1, 4 * G2], F32, tag="setup", bufs=2,
                              name="ps_rows")
            for r in range(4):
                nc.tensor.matmul(ps_rows[:, r * G2:(r + 1) * G2],
                                 gt_s[:, r:r + 1], st_sel[:], start=True,
                                 stop=False)
                nc.tensor.matmul(ps_rows[:, r * G2:(r + 1) * G2],
                                 dummyv[:, r:r + 1], notf[:], start=False,
                                 stop=True)
            nc.scalar.copy(rows4[:], ps_rows[:])
            for r, dst in enumerate([gx1b, gy1b, gx2b, gy2b]):
                ps = pp.tile([P, G2], F32, tag="setup", bufs=2, name="psbc")
                nc.tensor.matmul(ps[:], ones[:],
                                 rows4[:, r * G2:(r + 1) * G2],
                                 start=True, stop=True)
                nc.scalar.copy(dst[:], ps[:])

            # area_g broadcast: (gx2-gx1+1)*(gy2-gy1+1)
            wg = cp.tile([P, G2], F32)
            hg = cp.tile([P, G2], F32)
            nc.vector.scalar_tensor_tensor(wg[:], gx1b[:], -1.0, gx2b[:],
                                           op0=ALU.mult, op1=ALU.add)
            nc.vector.tensor_scalar_add(wg[:], wg[:], 1.0)
            nc.vector.scalar_tensor_tensor(hg[:], gy1b[:], -1.0, gy2b[:],
                                           op0=ALU.mult, op1=ALU.add)
            nc.vector.tensor_scalar_add(hg[:], hg[:], 1.0)
            nc.vector.tensor_tensor(area_gb[:], wg[:], hg[:], op=ALU.mult)

            # per-slot gather table: [gcx, gcy, ln gw, ln gh, cls+1]
            x1, y1 = gt_c[:, 0:1], gt_c[:, 1:2]
            x2, y2 = gt_c[:, 2:3], gt_c[:, 3:4]
            cls = gt_c[:, 4:5]
            gw = cp.tile([G2, 1], F32)
            gh = cp.tile([G2, 1], F32)
            nc.vector.scalar_tensor_tensor(gw[:], x1, -1.0, x2,
                                           op0=ALU.mult, op1=ALU.add)
            nc.vector.tensor_scalar_add(gw[:], gw[:], 1.0)
            nc.vector.scalar_tensor_tensor(gh[:], y1, -1.0, y2,
                                           op0=ALU.mult, op1=ALU.add)
            nc.vector.tensor_scalar_add(gh[:], gh[:], 1.0)
            nc.vector.scalar_tensor_tensor(gtv[:, 0:1], gw[:], 0.5, x1,
                                           op0=ALU.mult, op1=ALU.add)
            nc.vector.scalar_tensor_tensor(gtv[:, 1:2], gh[:], 0.5, y1,
                                           op0=ALU.mult, op1=ALU.add)
            nc.scalar.activation(gtv[:, 2:3], gw[:], ACTF.Ln)
            nc.scalar.activation(gtv[:, 3:4], gh[:], ACTF.Ln)
            nc.vector.tensor_scalar_add(gtv[:, 4:5], cls, 1.0)

            # ---- repeated body (reps>1 used only for slope timing) ----
            for _rep in range(reps):
              nc.sync.dma_start(anc_s[:], anc_ext.ap())
              av = anc_s[:].rearrange("p (t c) -> p t c", c=4)
              NCH = 4
              CH = NT // NCH
              for c_ in range(NCH):
                  cs = slice(c_ * CH, NT if c_ == NCH - 1 else (c_ + 1) * CH)
                  ax1v, ay1v = av[:, cs, 0], av[:, cs, 1]
                  ax2v, ay2v = av[:, cs, 2], av[:, cs, 3]
                  nc.vector.scalar_tensor_tensor(ew_all[:, cs], ax1v, -1.0,
                                                 ax2v, op0=ALU.mult,
                                                 op1=ALU.add)
                  nc.vector.tensor_scalar_add(ew_all[:, cs], ew_all[:, cs],
                                              1.0)
                  nc.vector.scalar_tensor_tensor(eh_all[:, cs], ay1v, -1.0,
                                                 ay2v, op0=ALU.mult,
                                                 op1=ALU.add)
                  nc.vector.tensor_scalar_add(eh_all[:, cs], eh_all[:, cs],
                                              1.0)
                  nc.gpsimd.tensor_tensor(area_a[:, cs], ew_all[:, cs],
                                          eh_all[:, cs], op=ALU.mult)
                  nc.vector.scalar_tensor_tensor(ecx_all[:, cs],
                                                 ew_all[:, cs], 0.5, ax1v,
                                                 op0=ALU.mult, op1=ALU.add)
                  nc.vector.scalar_tensor_tensor(ecy_all[:, cs],
                                                 eh_all[:, cs], 0.5, ay1v,
                                                 op0=ALU.mult, op1=ALU.add)
                  nc.vector.tensor_scalar_mul(nay1_all[:, cs], ay1v, -1.0)
                  nc.vector.reciprocal(iew_all[:, cs], ew_all[:, cs])
                  nc.vector.reciprocal(ieh_all[:, cs], eh_all[:, cs])
                  nc.scalar.activation(lew_all[:, cs], ew_all[:, cs], ACTF.Ln)
                  nc.scalar.activation(leh_all[:, cs], eh_all[:, cs], ACTF.Ln)

              # ------------------------------------------------ main loop
              TBMAX = kn["tbmax"]
              groups = []
              t0 = 0
              while t0 < NT:
                  tb = min(TBMAX, NT - t0)
                  groups.append((t0, tb))
                  t0 += tb
              def phase1(gt0, TB):
                  st = {}
                  st["ry1_st"] = wp.tile([P, TBMAX, G2], F32, tag="ry1_st",
                                         name="ry1_st", bufs=kn["sbufs"])
                  st["ry2_st"] = wp.tile([P, TBMAX, G2], F32, tag="ry2_st",
                                         name="ry2_st", bufs=kn["sbufs"])
                  st["iw0_st"] = wp.tile([P, TBMAX, G2], F32, tag="iw0_st",
                                         name="iw0_st", bufs=kn["sbufs"])
                  st["sy_st"] = wp.tile([P, TBMAX, G2], F32, tag="sy_st",
                                        name="sy_st", bufs=kn["sbufs"])
                  st["ih_st"] = wp.tile([P, TBMAX, G2], F32, tag="ih_st",
                                        name="ih_st", bufs=kn["sbufs"])
                  st["inter_st"] = wp.tile([P, TBMAX, G2], F32,
                                           tag="inter_st", name="inter_st",
                                           bufs=kn["sbufs"])
                  st["union_st"] = wp.tile([P, TBMAX, G2], F32,
                                           tag="union_st", name="union_st",
                                           bufs=kn["sbufs"])
                  st["iou_st"] = wp.tile([P, TBMAX, G2], F32, tag="iou_st",
                                         name="iou_st", bufs=kn["sbufs"])
                  st["mrev_st"] = wp.tile([P, TBMAX, G2], F32, tag="mrev_st",
                                          name="mrev_st", bufs=kn["sbufs"])
                  ry1_st, ry2_st = st["ry1_st"], st["ry2_st"]
                  iw0_st, sy_st, ih_st = st["iw0_st"], st["sy_st"], st["ih_st"]
                  inter_st, union_st = st["inter_st"], st["union_st"]
                  iou_st = st["iou_st"]
                  ydn = min(kn["y_dve_n"], TB)
                  st["ydn"] = ydn
                  for j in range(TB):
                      t = gt0 + j
                      ax1 = anc_s[:, 4 * t + 0:4 * t + 1]
                      ay1 = anc_s[:, 4 * t + 1:4 * t + 2]
                      ax2 = anc_s[:, 4 * t + 2:4 * t + 3]
                      ay2 = anc_s[:, 4 * t + 3:4 * t + 4]
                      if j < ydn:
                          t2y = wp.tile([P, G2], F32, tag="t2y", name="t2y")
                          nc.vector.tensor_scalar(t2y[:], gy1b[:], ay1, 1.0,
                                                  op0=ALU.max,
                                                  op1=ALU.subtract)
                          nc.vector.scalar_tensor_tensor(
                              ih_st[:, j, :], gy2b[:], ay2, t2y[:],
                              op0=ALU.min, op1=ALU.subtract)
                      else:
                          # y axis on ACT: two hinge terms
                          nc.scalar.activation(ry1_st[:, j, :], gy1b[:],
                                               ACTF.Relu,
                                               bias=nay1_all[:, t:t + 1])
                          nc.scalar.activation(ry2_st[:, j, :], gy2b[:],
                                               ACTF.Relu, bias=ay2,
                                               scale=-1.0)
                      # x axis on DVE
                      t2x = wp.tile([P, G2], F32, tag="t2x", name="t2x")
                      nc.vector.tensor_scalar(t2x[:], gx1b[:], ax1, 1.0,
                                              op0=ALU.max, op1=ALU.subtract)
                      nc.vector.scalar_tensor_tensor(iw0_st[:, j, :], gx2b[:],
                                                     ax2, t2x[:], op0=ALU.min,
                                                     op1=ALU.subtract)
                  bsl = (slice(None), slice(0, TB), slice(None))
                  asl = (slice(None), slice(ydn, TB), slice(None))
                  csl = (slice(None), slice(0, ydn), slice(None))

                  def _eng(key):
                      return nc.gpsimd if kn[key] == "P" else nc.vector
                  # batched: sy = ry1 + ry2 (ACT-y slices), iw clamp in place
                  if ydn < TB:
                      _eng("sy_eng").tensor_tensor(sy_st[asl], ry1_st[asl],
                                                   ry2_st[asl], op=ALU.add)
                  _eng("iw_eng").tensor_scalar(iw0_st[bsl], iw0_st[bsl], 0.0,
                                               None, op0=ALU.max)
                  for j in range(ydn, TB):
                      t = gt0 + j
                      # ih = relu(eh - sy) per tile on ACT
                      nc.scalar.activation(ih_st[:, j, :], sy_st[:, j, :],
                                           ACTF.Relu,
                                           bias=eh_all[:, t:t + 1],
                                           scale=-1.0)
                  if ydn:
                      # clamp ih for the DVE-y slices
                      _eng("ih_eng").tensor_scalar(ih_st[csl], ih_st[csl],
                                                   0.0, None, op0=ALU.max)
                  # batched: inter = iw * ih
                  _eng("inter_eng").tensor_tensor(inter_st[bsl], iw0_st[bsl],
                                                  ih_st[bsl], op=ALU.mult)
                  aa3 = (area_a[:, gt0:gt0 + TB]
                         .rearrange("p (t o) -> p t o", o=1)
                         .broadcast_to([P, TB, G2]))
                  ag3 = (area_gb[:]
                         .rearrange("p (o g) -> p o g", o=1)
                         .broadcast_to([P, TB, G2]))
                  _eng("asum_eng").tensor_tensor(union_st[bsl], aa3, ag3,
                                                 op=ALU.add)
                  _eng("union_eng").tensor_tensor(union_st[bsl],
                                                  union_st[bsl],
                                                  inter_st[bsl],
                                                  op=ALU.subtract)
                  # batched: urec = 1/union (DVE, in place), iou
                  nc.vector.reciprocal(union_st[bsl], union_st[bsl])
                  _eng("iou_eng").tensor_tensor(iou_st[bsl], inter_st[bsl],
                                                union_st[bsl], op=ALU.mult)
                  return st

              def phase2(gt0, TB, st):
                  bsl = (slice(None), slice(0, TB), slice(None))
                  iou_st, mrev_st = st["iou_st"], st["mrev_st"]
                  def _eng(key):
                      return nc.gpsimd if kn[key] == "P" else nc.vector
                  # batched max-iou over the group
                  nc.vector.tensor_reduce(minv_all[:, gt0:gt0 + TB],
                                          iou_st[bsl], axis=AX.X, op=ALU.max)
                  minv_b3 = (minv_all[:, gt0:gt0 + TB]
                             .rearrange("p (t o) -> p t o", o=1)
                             .broadcast_to([P, TB, G2]))
                  revp_b3 = (revp_b[:]
                             .rearrange("p (o g) -> p o g", o=1)
                             .broadcast_to([P, TB, G2]))
                  _eng("m0_eng").tensor_tensor(mrev_st[bsl], iou_st[bsl],
                                               minv_b3, op=ALU.is_equal)
                  _eng("mrevm_eng").tensor_tensor(mrev_st[bsl], mrev_st[bsl],
                                                  revp_b3, op=ALU.mult)
                  nc.vector.tensor_reduce(rmax_all[:, gt0:gt0 + TB],
                                          mrev_st[bsl], axis=AX.X, op=ALU.max)
                  rmax_b3 = (rmax_all[:, gt0:gt0 + TB]
                             .rearrange("p (t o) -> p t o", o=1)
                             .broadcast_to([P, TB, G2]))
                  oh_st = wp.tile([P, TBMAX, G2], F32, tag="oh_st",
                                  name="oh_st", bufs=kn["sbufs"])
                  _eng("oh_eng").tensor_tensor(oh_st[bsl], mrev_st[bsl],
                                               rmax_b3, op=ALU.is_equal)
                  ohT4 = wp.tile([G2, TBMAX * P], F32, tag="ohT4",
                                 name="ohT4", bufs=2)
                  gps4 = pp.tile([P, TBMAX * 5], F32, tag="gps4", name="gps4")
                  for h0 in range(0, TB, 4):
                      hn = min(4, TB - h0)
                      psT4 = pp.tile([G2, 4 * P], F32, tag="psT4",
                                     name="psT4")
                      for j in range(h0, h0 + hn):
                          nc.tensor.transpose(psT4[:, (j - h0) * P:
                                                   (j - h0 + 1) * P],
                                              oh_st[:, j, :], identity[:])
                      oc = kn["ohTcopy"]
                      if oc == "D" or (oc == "X" and (gt0 // TBMAX) % 2 == 0):
                          nc.vector.tensor_copy(ohT4[:, h0 * P:(h0 + hn) * P],
                                                psT4[:, 0:hn * P])
                      else:
                          nc.scalar.copy(ohT4[:, h0 * P:(h0 + hn) * P],
                                         psT4[:, 0:hn * P])
                  for j in range(TB):
                      nc.tensor.matmul(gps4[:, j * 5:(j + 1) * 5],
                                       ohT4[:, j * P:(j + 1) * P],
                                       gtv[:, 0:5], start=True, stop=True)
                  nc.scalar.copy(gath[:, 5 * gt0:5 * (gt0 + TB)],
                                 gps4[:, 0:TB * 5])

              # software-pipelined emission: phase1(g+1) before phase2(g)
              pend = []
              for (gt0, TB) in groups:
                  pend.append((gt0, TB, phase1(gt0, TB)))
                  if len(pend) > kn.get("skew", 1):
                      g0, tb0, st0 = pend.pop(0)
                      phase2(g0, tb0, st0)
              for (g0, tb0, st0) in pend:
                  phase2(g0, tb0, st0)
              # ------------------------------------------------ epilogue
              gv = gath[:].rearrange("p (t c) -> p t c", c=5)
              ob = outbuf[:].rearrange("p (t c) -> p t c", c=5)
              tmp1 = cp.tile([P, NT], F32)
              tmp2 = cp.tile([P, NT], F32)
              lm1 = cp.tile([P, NT], F32)
              lm2 = cp.tile([P, NT], F32)
              for c_ in range(NCH):
                  cs = slice(c_ * CH, NT if c_ == NCH - 1 else (c_ + 1) * CH)
                  # dx, dy
                  nc.vector.tensor_sub(tmp1[:, cs], gv[:, cs, 0],
                                       ecx_all[:, cs])
                  nc.vector.tensor_tensor(ob[:, cs, 1], tmp1[:, cs],
                                          iew_all[:, cs], op=ALU.mult)
                  nc.vector.tensor_sub(tmp2[:, cs], gv[:, cs, 1],
                                       ecy_all[:, cs])
                  nc.vector.tensor_tensor(ob[:, cs, 2], tmp2[:, cs],
                                          ieh_all[:, cs], op=ALU.mult)
                  # dw, dh
                  nc.vector.tensor_sub(ob[:, cs, 3], gv[:, cs, 2],
                                       lew_all[:, cs])
                  nc.vector.tensor_sub(ob[:, cs, 4], gv[:, cs, 3],
                                       leh_all[:, cs])
                  # labels: pos*(cls+1) + neg - 1
                  nc.vector.scalar_tensor_tensor(lm1[:, cs], minv_all[:, cs],
                                                 0.5, gv[:, cs, 4],
                                                 op0=ALU.is_ge, op1=ALU.mult)
                  nc.vector.scalar_tensor_tensor(lm2[:, cs], minv_all[:, cs],
                                                 0.4, lm1[:, cs],
                                                 op0=ALU.is_lt, op1=ALU.add)
                  nc.vector.tensor_scalar_add(ob[:, cs, 0], lm2[:, cs], -1.0)
              nc.sync.dma_start(out_ext.ap(), outbuf[:])

    nc.compile()
    return nc


_NC = {}


def _get_nc(reps: int = 1):
    if reps not in _NC:
        _NC[reps] = build_nc(reps)
    return _NC[reps]


def make_in_maps(gt_boxes):
    anchors = _shift(FW, FH, STRIDE, _generate_anchors(ANCHOR_SIZE))
    gt = np.asarray(gt_boxes, dtype=np.float32)[0]          # [G, 5]
    in_maps = []
    for c in range(NCORES):
        shard = anchors[c * NA:(c + 1) * NA]
        pad = np.zeros((NAP - NA, 4), dtype=np.float32)
        sh = np.concatenate([shard, pad], axis=0)           # [NAP, 4]
        anc = np.ascontiguousarray(
            sh.reshape(NT, P, 4).transpose(1, 0, 2).reshape(P, NT * 4))
        lo = np.float32(shard[:, 1].min() - 1.0)
        hi = np.float32(shard[:, 3].max() + 1.0)
        nkeep = 1 + int(np.sum((np.minimum(gt[1:, 3], hi)
                                - np.maximum(gt[1:, 1], lo) + 1.0) > 0))
        assert nkeep <= G2, f"core {c}: {nkeep} relevant gt boxes > G2={G2}"
        band = np.tile(np.array([[lo, hi]], dtype=np.float32), (G, 1))
        in_maps.append({"anc": anc, "gt": gt, "band": band})
    return anchors, in_maps


def kernel(gt_boxes, fw, fh):
    assert int(fw) == FW and int(fh) == FH
    anchors, in_maps = make_in_maps(gt_boxes)
    nc = _get_nc()
    res = run_bass_kernel_spmd(nc, in_maps, core_ids=list(range(NCORES)))
    parts = []
    for c in range(NCORES):
        o = res.results[c]["out"]                           # [P, NT*5]
        o = o.reshape(P, NT, 5).transpose(1, 0, 2).reshape(NAP, 5)[:NA]
        parts.append(o)
    full = np.concatenate(parts, axis=0)                    # [N, 5]
    labels = np.ascontiguousarray(full[:, 0])[None]
    targets = np.ascontiguousarray(full[:, 1:5])[None]
    return labels, targets, anchors[None]


if __name__ == "__main__":
    gt = np.random.rand(1, G, 5).astype(np.float32)
    out = kernel(gt_boxes=gt, fw=FW, fh=FH)
    print([o.shape for o in out])


# revision 28
# speedup vs baseline: 1.4319x; 1.0207x over previous
"""AnchorTarget kernel for 8 TRN2 NeuronCores (Bass/Tile).

Strategy: shard the anchor dimension N=273600 across 8 cores (34200 each,
padded to 268*128=34304); each core's anchors cover a contiguous 19-row
y-band of the feature grid. gt_boxes (100 boxes) are replicated, and each
core COMPACTS on-device the boxes that can overlap its band (box 0 is
always kept first so all-zero-IoU rows resolve to gt[0] like jnp.argmax)
into G2=40 slots via a prefix-sum rank (triangular-ones PE matmul) and a
one-hot selection matrix. The hot loop then computes IoU, first-index
argmax (reverse-iota trick) and the matched-gt gather (PE transpose +
matmul against a per-slot table [gcx, gcy, ln gw, ln gh, cls+1]) on
[128, 40] tiles, with per-op engine assignment balanced across
DVE/ACT/Pool and group-batched reductions.
"""

import sys

for _p in ("/opt/trn_rl_repo",):
    if _p not in sys.path:
        sys.path.insert(0, _p)

import numpy as np

import concourse.bass as bass
import concourse.bacc as bacc
import concourse.mybir as mybir
from concourse import tile, masks
from concourse.bass_utils import run_bass_kernel_spmd

F32 = mybir.dt.float32
ALU = mybir.AluOpType
ACTF = mybir.ActivationFunctionType
AX = mybir.AxisListType

STRIDE = 8
ANCHOR_SIZE = 32
FW, FH = 200, 152
G = 100
G2 = 40                    # compacted gt slots per core
A = 9
N = FW * FH * A            # 273600
NCORES = 8
NA = N // NCORES           # 34200 anchors per core
P = 128
NT = (NA + P - 1) // P     # 268 tiles per core
NAP = NT * P               # 34304 padded per core

RATIOS = np.array([0.5, 1.0, 2.0])
SCALES = np.array([2.0 ** 0, 2.0 ** (1.0 / 3.0), 2.0 ** (2.0 / 3.0)])

KNOBS = {"onehot": 2, "y_dve_n": 4, "wbufs": 8, "sbufs": 6, "skew": 3,
         "sy_eng": "P", "iw_eng": "P", "ih_eng": "P", "inter_eng": "P",
         "iou_eng": "P", "tbmax": 8, "ohTcopy": "A",
         "asum_eng": "D", "union_eng": "D",
         "m0_eng": "D", "mrevm_eng": "P", "oh_eng": "D"}


# ---------------------------------------------------------------- host anchors
def _whctrs(a):
    w = a[2] - a[0] + 1.0
    h = a[3] - a[1] + 1.0
    xc = a[0] + 0.5 * (w - 1.0)
    yc = a[1] + 0.5 * (h - 1.0)
    return w, h, xc, yc


def _mkanchors(ws, hs, xc, yc):
    ws = ws[:, None]
    hs = hs[:, None]
    return np.hstack([xc - 0.5 * (ws - 1.0), yc - 0.5 * (hs - 1.0),
                      xc + 0.5 * (ws - 1.0), yc + 0.5 * (hs - 1.0)])


def _generate_anchors(base_size):
    base = np.array([0.0, 0.0, base_size - 1.0, base_size - 1.0])
    w, h, xc, yc = _whctrs(base)
    size_ratios = (w * h) / RATIOS
    ws = np.round(np.sqrt(size_ratios))
    hs = np.round(ws * RATIOS)
    ratio_anchors = _mkanchors(ws, hs, xc, yc)
    out = []
    for i in range(ratio_anchors.shape[0]):
        w, h, xc, yc = _whctrs(ratio_anchors[i])
        out.append(_mkanchors(w * SCALES, h * SCALES, xc, yc))
    return np.vstack(out).astype(np.float32)


def _shift(fw, fh, stride, anchors):
    sx = np.arange(fw) * stride
    sy = np.arange(fh) * stride
    sx, sy = np.meshgrid(sx, sy)
    shifts = np.stack([sx.ravel(), sy.ravel(), sx.ravel(), sy.ravel()], axis=1)
    alla = anchors[None, :, :] + shifts[:, None, :].astype(np.float32)
    return alla.reshape(-1, 4).astype(np.float32)


# ---------------------------------------------------------------- device graph
def build_nc(reps: int = 1):
    kn = KNOBS
    nc = bacc.Bacc("TRN2", target_bir_lowering=False, debug=False,
                   num_devices=NCORES)

    anc_ext = nc.dram_tensor("anc", [P, NT * 4], F32, kind="ExternalInput")
    gt_ext = nc.dram_tensor("gt", [G, 5], F32, kind="ExternalInput")
    band_ext = nc.dram_tensor("band", [G, 2], F32, kind="ExternalInput")
    out_ext = nc.dram_tensor("out", [P, NT * 5], F32, kind="ExternalOutput")

    with tile.TileContext(nc) as tc:
        from contextlib import ExitStack
        ctx = ExitStack()
        with ctx:
            cp = ctx.enter_context(tc.tile_pool(name="const", bufs=1))
            wp = ctx.enter_context(
                tc.tile_pool(name="work", bufs=kn["wbufs"]))
            pp = ctx.enter_context(tc.tile_pool(name="psum", bufs=3,
                                                space="PSUM"))

            # persistent buffers
            anc_s = cp.tile([P, NT * 4], F32)
            outbuf = cp.tile([P, NT * 5], F32)
            gath = cp.tile([P, NT * 5], F32)
            minv_all = cp.tile([P, NT], F32)
            rmax_all = cp.tile([P, NT], F32)
            onebias_all = cp.tile([P, NT], F32)
            gt_s = cp.tile([G, 5], F32)
            band_s = cp.tile([G, 2], F32)
            identity = cp.tile([P, P], F32)
            ltri = cp.tile([G, G], F32)
            ones = cp.tile([1, P], F32)
            slotids = cp.tile([P, G2], F32)
            gtv = cp.tile([G2, 5], F32)
            gt_c = cp.tile([G2, 5], F32)
            rows4 = cp.tile([1, 4 * G2], F32)
            dummyv = cp.tile([1, 4], F32)
            keep = cp.tile([G, 1], F32)
            rank = cp.tile([G, 1], F32)
            rank_m = cp.tile([G, 1], F32)
            notf = cp.tile([1, G2], F32)
            st_sel = cp.tile([G, G2], F32)
            gx1b = cp.tile([P, G2], F32)
            gy1b = cp.tile([P, G2], F32)
            gx2b = cp.tile([P, G2], F32)
            gy2b = cp.tile([P, G2], F32)
            area_gb = cp.tile([P, G2], F32)
            revp_b = cp.tile([P, G2], F32)
            # per-anchor batched quantities
            ew_all = cp.tile([P, NT], F32)
            eh_all = cp.tile([P, NT], F32)
            area_a = cp.tile([P, NT], F32)
            ecx_all = cp.tile([P, NT], F32)
            ecy_all = cp.tile([P, NT], F32)
            iew_all = cp.tile([P, NT], F32)
            ieh_all = cp.tile([P, NT], F32)
            lew_all = cp.tile([P, NT], F32)
            leh_all = cp.tile([P, NT], F32)
            nay1_all = cp.tile([P, NT], F32)

            # input DMAs
            nc.sync.dma_start(gt_s[:], gt_ext.ap())
            nc.sync.dma_start(band_s[:], band_ext.ap())

            # constants: identity, lower-tri ones, ones row, slot iota
            masks.make_identity(nc, identity[:])
            nc.gpsimd.memset(ltri[:], 0.0)
            nc.gpsimd.affine_select(
                out=ltri[:], in_=ltri[:], compare_op=ALU.is_gt, fill=1.0,
                base=0, pattern=[[-1, G]], channel_multiplier=1)
            nc.gpsimd.memset(ones[:], 1.0)
            slot_i = cp.tile([P, G2], mybir.dt.int32)
            nc.gpsimd.iota(slot_i[:], pattern=[[1, G2]], base=0,
                           channel_multiplier=0)
            nc.vector.tensor_copy(slotids[:], slot_i[:])
            revp_i = cp.tile([P, G2], mybir.dt.int32)
            nc.gpsimd.iota(revp_i[:], pattern=[[-1, G2]], base=G2,
                           channel_multiplier=0)
            nc.vector.tensor_copy(revp_b[:], revp_i[:])
            nc.gpsimd.memset(dummyv[:, 0:2], -100000.0)
            nc.gpsimd.memset(dummyv[:, 2:4], -99999.0)

            # ---- gt band compaction ----
            gy1, gy2 = gt_s[:, 1:2], gt_s[:, 3:4]
            blo, bhi = band_s[:, 0:1], band_s[:, 1:2]
            m1 = cp.tile([G, 1], F32)
            m2 = cp.tile([G, 1], F32)
            nc.vector.tensor_tensor(m1[:], gy2, bhi, op=ALU.min)
            nc.vector.tensor_tensor(m2[:], gy1, blo, op=ALU.max)
            k0 = cp.tile([G, 1], F32)
            nc.vector.scalar_tensor_tensor(k0[:], m1[:], 1.0, m2[:],
                                           op0=ALU.add, op1=ALU.subtract)
            nc.vector.tensor_scalar(keep[:], k0[:], 0.0, None, op0=ALU.is_gt)
            nc.gpsimd.memset(keep[0:1, :], 1.0)
            # inclusive prefix sum of keep via lower-triangular ones
            ps_rank = pp.tile([G, 1], F32, tag="setup", bufs=2, name="ps_rank")
            nc.tensor.matmul(ps_rank[:], ltri[:], keep[:], start=True,
                             stop=True)
            nc.scalar.copy(rank[:], ps_rank[:])
            # slot = rank-1 for kept, >=999 for dropped
            a999 = cp.tile([G, 1], F32)
            nc.vector.tensor_scalar_add(a999[:], rank[:], 999.0)
            nc.vector.scalar_tensor_tensor(rank_m[:], keep[:], -1000.0,
                                           a999[:], op0=ALU.mult, op1=ALU.add)
            # selection matrix [g, slot] and unfilled-slot row
            nc.vector.tensor_scalar(st_sel[:], slotids[0:G, :], rank_m[:],
                                    None, op0=ALU.is_equal)
            ones100 = cp.tile([G, 1], F32)
            nc.gpsimd.memset(ones100[:], 1.0)
            ps_cnt = pp.tile([1, 1], F32, tag="setup", bufs=2, name="ps_cnt")
            nc.tensor.matmul(ps_cnt[:], keep[:], ones100[:], start=True,
                             stop=True)
            cnt = cp.tile([1, 1], F32)
            nc.scalar.copy(cnt[:], ps_cnt[:])
            nc.vector.tensor_scalar(notf[:], slotids[0:1, :], cnt[:, 0:1],
                                    None, op0=ALU.is_ge)
            # compacted gt rows [G2, 5] (+dummy box into unfilled slots)
            ps_gtc = pp.tile([G2, 5], F32, tag="setup", bufs=2, name="ps_gtc")
            nc.tensor.matmul(ps_gtc[:], st_sel[:], gt_s[:], start=True,
                             stop=False)
            dummy5 = cp.tile([1, 5], F32)
            nc.vector.tensor_copy(dummy5[:, 0:4], dummyv[:])
            nc.gpsimd.memset(dummy5[:, 4:5], 0.0)
            nc.tensor.matmul(ps_gtc[:], notf[:], dummy5[:], start=False,
                             stop=True)
            nc.scalar.copy(gt_c[:], ps_gtc[:])
            # compacted coord rows [1, 4*G2] then broadcast to [128, G2]
            ps_rows = pp.tile([1, 4 * G2], F32, tag="setup", bufs=2,
                              name="ps_rows")
            for r in range(4):
                nc.tensor.matmul(ps_rows[:, r * G2:(r + 1) * G2],
                                 gt_s[:, r:r + 1], st_sel[:], start=True,
                                 stop=False)
                nc.tensor.matmul(ps_rows[:, r * G2:(r + 1) * G2],
                                 dummyv[:, r:r + 1], notf[:], start=False,
                                 stop=True)
            nc.scalar.copy(rows4[:], ps_rows[:])
            for r, dst in enumerate([gx1b, gy1b, gx2b, gy2b]):
                ps = pp.tile([P, G2], F32, tag="setup", bufs=2, name="psbc")
                nc.tensor.matmul(ps[:], ones[:],
                                 rows4[:, r * G2:(r + 1) * G2],
                                 start=True, stop=True)
                nc.scalar.copy(dst[:], ps[:])

            # area_g broadcast: (gx2-gx1+1)*(gy2-gy1+1)
            wg = cp.tile([P, G2], F32)
            hg = cp.tile([P, G2], F32)
            nc.vector.scalar_tensor_tensor(wg[:], gx1b[:], -1.0, gx2b[:],
                                           op0=ALU.mult, op1=ALU.add)
            nc.vector.tensor_scalar_add(wg[:], wg[:], 1.0)
            nc.vector.scalar_tensor_tensor(hg[:], gy1b[:], -1.0, gy2b[:],
                                           op0=ALU.mult, op1=ALU.add)
            nc.vector.tensor_scalar_add(hg[:], hg[:], 1.0)
            nc.vector.tensor_tensor(area_gb[:], wg[:], hg[:], op=ALU.mult)

            # per-slot gather table: [gcx, gcy, ln gw, ln gh, cls+1]
            x1, y1 = gt_c[:, 0:1], gt_c[:, 1:2]
            x2, y2 = gt_c[:, 2:3], gt_c[:, 3:4]
            cls = gt_c[:, 4:5]
            gw = cp.tile([G2, 1], F32)
            gh = cp.tile([G2, 1], F32)
            nc.vector.scalar_tensor_tensor(gw[:], x1, -1.0, x2,
                                           op0=ALU.mult, op1=ALU.add)
            nc.vector.tensor_scalar_add(gw[:], gw[:], 1.0)
            nc.vector.scalar_tensor_tensor(gh[:], y1, -1.0, y2,
                                           op0=ALU.mult, op1=ALU.add)
            nc.vector.tensor_scalar_add(gh[:], gh[:], 1.0)
            nc.vector.scalar_tensor_tensor(gtv[:, 0:1], gw[:], 0.5, x1,
                                           op0=ALU.mult, op1=ALU.add)
            nc.vector.scalar_tensor_tensor(gtv[:, 1:2], gh[:], 0.5, y1,
                                           op0=ALU.mult, op1=ALU.add)
            nc.scalar.activation(gtv[:, 2:3], gw[:], ACTF.Ln)
            nc.scalar.activation(gtv[:, 3:4], gh[:], ACTF.Ln)
            nc.vector.tensor_scalar_add(gtv[:, 4:5], cls, 1.0)

            # ---- repeated body (reps>1 used only for slope timing) ----
            for _rep in range(reps):
              nc.sync.dma_start(anc_s[:], anc_ext.ap())
              av = anc_s[:].rearrange("p (t c) -> p t c", c=4)
              NCH = 4
              CH = NT // NCH
              for c_ in range(NCH):
                  cs = slice(c_ * CH, NT if c_ == NCH - 1 else (c_ + 1) * CH)
                  ax1v, ay1v = av[:, cs, 0], av[:, cs, 1]
                  ax2v, ay2v = av[:, cs, 2], av[:, cs, 3]
                  nc.vector.scalar_tensor_tensor(ew_all[:, cs], ax1v, -1.0,
                                                 ax2v, op0=ALU.mult,
                                                 op1=ALU.add)
                  nc.vector.tensor_scalar_add(ew_all[:, cs], ew_all[:, cs],
                                              1.0)
                  nc.vector.scalar_tensor_tensor(eh_all[:, cs], ay1v, -1.0,
                                                 ay2v, op0=ALU.mult,
                                                 op1=ALU.add)
                  nc.vector.tensor_scalar_add(eh_all[:, cs], eh_all[:, cs],
                                              1.0)
                  nc.gpsimd.tensor_tensor(area_a[:, cs], ew_all[:, cs],
                                          eh_all[:, cs], op=ALU.mult)
                  nc.vector.scalar_tensor_tensor(ecx_all[:, cs],
                                                 ew_all[:, cs], 0.5, ax1v,
                                                 op0=ALU.mult, op1=ALU.add)
                  nc.vector.scalar_tensor_tensor(ecy_all[:, cs],
                                                 eh_all[:, cs], 0.5, ay1v,
                                                 op0=ALU.mult, op1=ALU.add)
                  nc.vector.tensor_scalar_mul(nay1_all[:, cs], ay1v, -1.0)
                  nc.vector.reciprocal(iew_all[:, cs], ew_all[:, cs])
                  nc.vector.reciprocal(ieh_all[:, cs], eh_all[:, cs])
                  nc.scalar.activation(lew_all[:, cs], ew_all[:, cs], ACTF.Ln)
                  nc.scalar.activation(leh_all[:, cs], eh_all[:, cs], ACTF.Ln)

              # ------------------------------------------------ main loop
              TBMAX = kn["tbmax"]
              groups = []
              t0 = 0
              while t0 < NT:
                  tb = min(TBMAX, NT - t0)
                  groups.append((t0, tb))
                  t0 += tb
              def phase1(gt0, TB):
                  st = {}
                  st["ry1_st"] = wp.tile([P, TBMAX, G2], F32, tag="ry1_st",
                                         name="ry1_st", bufs=kn["sbufs"])
                  st["ry2_st"] = wp.tile([P, TBMAX, G2], F32, tag="ry2_st",
                                         name="ry2_st", bufs=kn["sbufs"])
                  st["iw0_st"] = wp.tile([P, TBMAX, G2], F32, tag="iw0_st",
                                         name="iw0_st", bufs=kn["sbufs"])
                  st["sy_st"] = wp.tile([P, TBMAX, G2], F32, tag="sy_st",
                                        name="sy_st", bufs=kn["sbufs"])
                  st["ih_st"] = wp.tile([P, TBMAX, G2], F32, tag="ih_st",
                                        name="ih_st", bufs=kn["sbufs"])
                  st["inter_st"] = wp.tile([P, TBMAX, G2], F32,
                                           tag="inter_st", name="inter_st",
                                           bufs=kn["sbufs"])
                  st["union_st"] = wp.tile([P, TBMAX, G2], F32,
                                           tag="union_st", name="union_st",
                                           bufs=kn["sbufs"])
                  st["iou_st"] = wp.tile([P, TBMAX, G2], F32, tag="iou_st",
                                         name="iou_st", bufs=kn["sbufs"])
                  st["mrev_st"] = wp.tile([P, TBMAX, G2], F32, tag="mrev_st",
                                          name="mrev_st", bufs=kn["sbufs"])
                  ry1_st, ry2_st = st["ry1_st"], st["ry2_st"]
                  iw0_st, sy_st, ih_st = st["iw0_st"], st["sy_st"], st["ih_st"]
                  inter_st, union_st = st["inter_st"], st["union_st"]
                  iou_st = st["iou_st"]
                  ydn = min(kn["y_dve_n"], TB)
                  st["ydn"] = ydn
                  for j in range(TB):
                      t = gt0 + j
                      ax1 = anc_s[:, 4 * t + 0:4 * t + 1]
                      ay1 = anc_s[:, 4 * t + 1:4 * t + 2]
                      ax2 = anc_s[:, 4 * t + 2:4 * t + 3]
                      ay2 = anc_s[:, 4 * t + 3:4 * t + 4]
                      if j < ydn:
                          t2y = wp.tile([P, G2], F32, tag="t2y", name="t2y")
                          nc.vector.tensor_scalar(t2y[:], gy1b[:], ay1, 1.0,
                                                  op0=ALU.max,
                                                  op1=ALU.subtract)
                          nc.vector.scalar_tensor_tensor(
                              ih_st[:, j, :], gy2b[:], ay2, t2y[:],
                              op0=ALU.min, op1=ALU.subtract)
                      else:
                          # y axis on ACT: two hinge terms
                          nc.scalar.activation(ry1_st[:, j, :], gy1b[:],
                                               ACTF.Relu,
                                               bias=nay1_all[:, t:t + 1])
                          nc.scalar.activation(ry2_st[:, j, :], gy2b[:],
                                               ACTF.Relu, bias=ay2,
                                               scale=-1.0)
                      # x axis on DVE
                      t2x = wp.tile([P, G2], F32, tag="t2x", name="t2x")
                      nc.vector.tensor_scalar(t2x[:], gx1b[:], ax1, 1.0,
                                              op0=ALU.max, op1=ALU.subtract)
                      nc.vector.scalar_tensor_tensor(iw0_st[:, j, :], gx2b[:],
                                                     ax2, t2x[:], op0=ALU.min,
                                                     op1=ALU.subtract)
                  bsl = (slice(None), slice(0, TB), slice(None))
                  asl = (slice(None), slice(ydn, TB), slice(None))
                  csl = (slice(None), slice(0, ydn), slice(None))

                  def _eng(key):
                      return nc.gpsimd if kn[key] == "P" else nc.vector
                  # batched: sy = ry1 + ry2 (ACT-y slices), iw clamp in place
                  if ydn < TB:
                      _eng("sy_eng").tensor_tensor(sy_st[asl], ry1_st[asl],
                                                   ry2_st[asl], op=ALU.add)
                  _eng("iw_eng").tensor_scalar(iw0_st[bsl], iw0_st[bsl], 0.0,
                                               None, op0=ALU.max)
                  for j in range(ydn, TB):
                      t = gt0 + j
                      # ih = relu(eh - sy) per tile on ACT
                      nc.scalar.activation(ih_st[:, j, :], sy_st[:, j, :],
                                           ACTF.Relu,
                                           bias=eh_all[:, t:t + 1],
                                           scale=-1.0)
                  if ydn:
                      # clamp ih for the DVE-y slices
                      _eng("ih_eng").tensor_scalar(ih_st[csl], ih_st[csl],
                                                   0.0, None, op0=ALU.max)
                  # batched: inter = iw * ih
                  _eng("inter_eng").tensor_tensor(inter_st[bsl], iw0_st[bsl],
                                                  ih_st[bsl], op=ALU.mult)
                  aa3 = (area_a[:, gt0:gt0 + TB]
                         .rearrange("p (t o) -> p t o", o=1)
                         .broadcast_to([P, TB, G2]))
                  ag3 = (area_gb[:]
                         .rearrange("p (o g) -> p o g", o=1)
                         .broadcast_to([P, TB, G2]))
                  _eng("asum_eng").tensor_tensor(union_st[bsl], aa3, ag3,
                                                 op=ALU.add)
                  _eng("union_eng").tensor_tensor(union_st[bsl],
                                                  union_st[bsl],
                                                  inter_st[bsl],
                                                  op=ALU.subtract)
                  # batched: urec = 1/union (DVE, in place), iou
                  nc.vector.reciprocal(union_st[bsl], union_st[bsl])
                  _eng("iou_eng").tensor_tensor(iou_st[bsl], inter_st[bsl],
                                                union_st[bsl], op=ALU.mult)
                  return st

              def phase2(gt0, TB, st):
                  bsl = (slice(None), slice(0, TB), slice(None))
                  iou_st, mrev_st = st["iou_st"], st["mrev_st"]
                  def _eng(key):
                      return nc.gpsimd if kn[key] == "P" else nc.vector
                  # batched max-iou over the group
                  nc.vector.tensor_reduce(minv_all[:, gt0:gt0 + TB],
                                          iou_st[bsl], axis=AX.X, op=ALU.max)
                  minv_b3 = (minv_all[:, gt0:gt0 + TB]
                             .rearrange("p (t o) -> p t o", o=1)
                             .broadcast_to([P, TB, G2]))
                  revp_b3 = (revp_b[:]
                             .rearrange("p (o g) -> p o g", o=1)
                             .broadcast_to([P, TB, G2]))
                  _eng("m0_eng").tensor_tensor(mrev_st[bsl], iou_st[bsl],
                                               minv_b3, op=ALU.is_equal)
                  _eng("mrevm_eng").tensor_tensor(mrev_st[bsl], mrev_st[bsl],
                                                  revp_b3, op=ALU.mult)
                  nc.vector.tensor_reduce(rmax_all[:, gt0:gt0 + TB],
                                          mrev_st[bsl], axis=AX.X, op=ALU.max)
                  rmax_b3 = (rmax_all[:, gt0:gt0 + TB]
                             .rearrange("p (t o) -> p t o", o=1)
                             .broadcast_to([P, TB, G2]))
                  oh_st = wp.tile([P, TBMAX, G2], F32, tag="oh_st",
                                  name="oh_st", bufs=kn["sbufs"])
                  _eng("oh_eng").tensor_tensor(oh_st[bsl], mrev_st[bsl],
                                               rmax_b3, op=ALU.is_equal)
                  ohT4 = wp.tile([G2, TBMAX * P], F32, tag="ohT4",
                                 name="ohT4", bufs=2)
                  gps4 = pp.tile([P, TBMAX * 5], F32, tag="gps4", name="gps4")
                  for h0 in range(0, TB, 4):
                      hn = min(4, TB - h0)
                      psT4 = pp.tile([G2, 4 * P], F32, tag="psT4",
                                     name="psT4")
                      for j in range(h0, h0 + hn):
                          nc.tensor.transpose(psT4[:, (j - h0) * P:
                                                   (j - h0 + 1) * P],
                                              oh_st[:, j, :], identity[:])
                      oc = kn["ohTcopy"]
                      if oc == "D" or (oc == "X" and (gt0 // TBMAX) % 2 == 0):
                          nc.vector.tensor_copy(ohT4[:, h0 * P:(h0 + hn) * P],
                                                psT4[:, 0:hn * P])
                      else:
                          nc.scalar.copy(ohT4[:, h0 * P:(h0 + hn) * P],
                                         psT4[:, 0:hn * P])
                  for j in range(TB):
                      nc.tensor.matmul(gps4[:, j * 5:(j + 1) * 5],
                                       ohT4[:, j * P:(j + 1) * P],
                                       gtv[:, 0:5], start=True, stop=True)
                  nc.scalar.copy(gath[:, 5 * gt0:5 * (gt0 + TB)],
                                 gps4[:, 0:TB * 5])

              # software-pipelined emission: phase1(g+1) before phase2(g)
              pend = []
              for (gt0, TB) in groups:
                  pend.append((gt0, TB, phase1(gt0, TB)))
                  if len(pend) > kn.get("skew", 1):
                      g0, tb0, st0 = pend.pop(0)
                      phase2(g0, tb0, st0)
              for (g0, tb0, st0) in pend:
                  phase2(g0, tb0, st0)
              # ------------------------------------------------ epilogue
              gv = gath[:].rearrange("p (t c) -> p t c", c=5)
              ob = outbuf[:].rearrange("p (t c) -> p t c", c=5)
              tmp1 = cp.tile([P, NT], F32)
              tmp2 = cp.tile([P, NT], F32)
              lm1 = cp.tile([P, NT], F32)
              lm2 = cp.tile([P, NT], F32)
              for c_ in range(NCH):
                  cs = slice(c_ * CH, NT if c_ == NCH - 1 else (c_ + 1) * CH)
                  # dx, dy
                  nc.vector.tensor_sub(tmp1[:, cs], gv[:, cs, 0],
                                       ecx_all[:, cs])
                  nc.vector.tensor_tensor(ob[:, cs, 1], tmp1[:, cs],
                                          iew_all[:, cs], op=ALU.mult)
                  nc.vector.tensor_sub(tmp2[:, cs], gv[:, cs, 1],
                                       ecy_all[:, cs])
                  nc.vector.tensor_tensor(ob[:, cs, 2], tmp2[:, cs],
                                          ieh_all[:, cs], op=ALU.mult)
                  # dw, dh
                  nc.vector.tensor_sub(ob[:, cs, 3], gv[:, cs, 2],
                                       lew_all[:, cs])
                  nc.vector.tensor_sub(ob[:, cs, 4], gv[:, cs, 3],
                                       leh_all[:, cs])
                  # labels: pos*(cls+1) + neg - 1
                  nc.vector.scalar_tensor_tensor(lm1[:, cs], minv_all[:, cs],
                                                 0.5, gv[:, cs, 4],
                                                 op0=ALU.is_ge, op1=ALU.mult)
                  nc.vector.scalar_tensor_tensor(lm2[:, cs], minv_all[:, cs],
                                                 0.4, lm1[:, cs],
                                                 op0=ALU.is_lt, op1=ALU.add)
                  nc.vector.tensor_scalar_add(ob[:, cs, 0], lm2[:, cs], -1.0)
              nc.sync.dma_start(out_ext.ap(), outbuf[:])

    nc.compile()
    return nc


_NC = {}


def _get_nc(reps: int = 1):
    if reps not in _NC:
        _NC[reps] = build_nc(reps)
    return _NC[reps]


def make_in_maps(gt_boxes):
    anchors = _shift(FW, FH, STRIDE, _generate_anchors(ANCHOR_SIZE))
    gt = np.asarray(gt_boxes, dtype=np.float32)[0]          # [G, 5]
    in_maps = []
    for c in range(NCORES):
        shard = anchors[c * NA:(c + 1) * NA]
        pad = np.zeros((NAP - NA, 4), dtype=np.float32)
        sh = np.concatenate([shard, pad], axis=0)           # [NAP, 4]
        anc = np.ascontiguousarray(
            sh.reshape(NT, P, 4).transpose(1, 0, 2).reshape(P, NT * 4))
        lo = np.float32(shard[:, 1].min() - 1.0)
        hi = np.float32(shard[:, 3].max() + 1.0)
        nkeep = 1 + int(np.sum((np.minimum(gt[1:, 3], hi)
                                - np.maximum(gt[1:, 1], lo) + 1.0) > 0))
        assert nkeep <= G2, f"core {c}: {nkeep} relevant gt boxes > G2={G2}"
        band = np.tile(np.array([[lo, hi]], dtype=np.float32), (G, 1))
        in_maps.append({"anc": anc, "gt": gt, "band": band})
    return anchors, in_maps


def kernel(gt_boxes, fw, fh):
    assert int(fw) == FW and int(fh) == FH
    anchors, in_maps = make_in_maps(gt_boxes)
    nc = _get_nc()
    res = run_bass_kernel_spmd(nc, in_maps, core_ids=list(range(NCORES)))
    parts = []
    for c in range(NCORES):
        o = res.results[c]["out"]                           # [P, NT*5]
        o = o.reshape(P, NT, 5).transpose(1, 0, 2).reshape(NAP, 5)[:NA]
        parts.append(o)
    full = np.concatenate(parts, axis=0)                    # [N, 5]
    labels = np.ascontiguousarray(full[:, 0])[None]
    targets = np.ascontiguousarray(full[:, 1:5])[None]
    return labels, targets, anchors[None]


if __name__ == "__main__":
    gt = np.random.rand(1, G, 5).astype(np.float32)
    out = kernel(gt_boxes=gt, fw=FW, fh=FH)
    print([o.shape for o in out])
